# revision 1
# baseline (speedup 1.0000x reference)
"""ARMA-style GNN message passing on 8 TRN2 NeuronCores.

Reference computation (per layer, 7 layers):
    m   = h @ W                                  [N, CH]
    agg = segment_sum(w[:,None] * m[dst], src)   [N, CH]
    h'  = relu(agg + h @ V + b)
then logits = h @ Wd + bd.

Strategy (graph/data parallel over nodes):
  - 8 cores own 1250 nodes each (padded to 1280 = 10 blocks of 128).
  - Edge (s, d) is processed by the core owning s (the aggregation target).
    Host sorts each core's edges into its src node-blocks, pads each block's
    edge list to a multiple of 128, and builds per-edge-block:
      * gather indices (padded-global row of d in the all-gathered m table)
      * a [128 edges x 128 nodes] bf16 "selection" matrix carrying the
        degree weights w_e -- segment-sum becomes sel.T @ gathered_rows on PE.
  - Per layer: each core computes m for its own nodes (PE), AllGathers m
    (bf16) so every core has the full table in DRAM, then per edge block
    one indirect DMA gathers 128 rows (1 KB each) and one matmul
    scatter-adds them into the PSUM accumulator of the owning node block.
    h@V accumulates into the same PSUM bank; bias + relu + PE-transpose
    produce the next layer's stationary operand hT.
  - Final dense layer and output assembly per core; host concatenates.

All matmuls run in bf16 with fp32 PSUM accumulation.
"""
import numpy as np
import ml_dtypes

import concourse.bass as bass
import concourse.tile as tile
import concourse.mybir as mybir
from concourse.vector_clock import ScopedClock
from concourse.bass_utils import run_bass_kernel_spmd
from concourse.masks import make_identity

# ---------------------------------------------------------------- constants
N_NODES = 10000
N_EDGES = 160000
IN_F = 256
CH = 512
N_LABELS = 1440
NCORES = 8
NPC = N_NODES // NCORES      # 1250 nodes per core
P = 128
NBL = 10                     # node blocks per core (10*128 = 1280)
NPAD = NBL * P               # padded nodes per core
NLAYERS = 7
KG1 = IN_F // P              # 2 contraction blocks in layer 1
KGC = CH // P                # 4 contraction blocks in layers 2..7
FIN_CHUNK = 480              # 1440 = 3 * 480, fits one PSUM bank in f32

BF = mybir.dt.bfloat16
F32 = mybir.dt.float32
BFNP = ml_dtypes.bfloat16


# ------------------------------------------------------- walrus workarounds
def _patched_drain_and_barrier(self, tick_clock, wait_clock):
    # This walrus build rejects >1-2 sync waits on one TPB_CTRL; put the
    # kernel-tail drain's waits on separate preceding SP nops instead.
    nc = self.nc
    probe = nc.sync.nop(nofuse=True, hint="drain_waits")
    wait_clock.add_sem_waits(probe.ins, ScopedClock({None: tick_clock.global_clock}))
    si = probe.ins.sync_info
    waits = list(si.on_wait) if si is not None else []
    if len(waits) > 1:
        si.on_wait = waits[:1]
        for i in range(1, len(waits)):
            n2 = nc.sync.nop(nofuse=True, hint=f"drain_waits_{i}")
            n2.ins.sync_info = mybir.SyncInfo(on_wait=[waits[i]], on_update=[])
    nc.sync.drain()
    nc.all_engine_barrier()
    assert self.sems is not None
    popped = nc._tile_sem_poison_stack.pop()
    assert popped is self._sem_poison
    nc.clear_and_free_semaphores(list(self.sems.allocated().values()))
    nc.all_engine_barrier()


tile.TileContext._drain_and_barrier = _patched_drain_and_barrier


def _split_excess_waits(nc, limit=1):
    # Same ISA restriction for ordinary instructions: hoist excess sync
    # waits onto injected same-engine nops placed just before.
    for func in nc.m.functions:
        for bb in func.blocks:
            out = []
            for ins in bb.instructions:
                si = ins.sync_info
                if si is not None and si.on_wait and len(si.on_wait) > limit:
                    waits = list(si.on_wait)
                    excess, keep = waits[:-limit], waits[-limit:]
                    for i in range(0, len(excess), limit):
                        out.append(mybir.InstNoOp(
                            name=f"{ins.name}_xw{i}",
                            engine=ins.engine,
                            ins=[], outs=[],
                            sync_info=mybir.SyncInfo(
                                on_wait=excess[i:i + limit], on_update=[]),
                        ))
                    si.on_wait = keep
                out.append(ins)
            bb.instructions[:] = out


# ------------------------------------------------------------- host prep
def _prep_edges(src, dst):
    """Partition/sort edges by owning core of src; build per-core gather
    index tables and selection matrices. Returns (knb, idx_tabs, sel_tabs)
    where knb[nb] = edge-block count of node block nb (same on all cores)."""
    src = np.asarray(src).astype(np.int64)
    dst = np.asarray(dst).astype(np.int64)
    deg_out = np.maximum(np.bincount(src, minlength=N_NODES), 1.0).astype(np.float32)
    deg_in = np.maximum(np.bincount(dst, minlength=N_NODES), 1.0).astype(np.float32)
    w = 1.0 / np.sqrt(deg_out[src] * deg_in[dst])

    core = src // NPC
    loc = src - core * NPC
    nb = loc // P
    ncol = loc - nb * P          # column within the node block's sel matrix
    grow = (dst // NPC) * NPAD + (dst % NPC)   # padded-global gather row

    # bucket edges by (core, nb)
    order = np.lexsort((nb, core))
    core_s, nb_s = core[order], nb[order]
    ncol_s, grow_s, w_s = ncol[order], grow[order], w[order]
    counts = np.zeros((NCORES, NBL), np.int64)
    np.add.at(counts, (core_s, nb_s), 1)
    knb = [max(1, int(-(-counts[:, b].max() // P))) for b in range(NBL)]
    neb = sum(knb)

    idx_tabs, sel_tabs = [], []
    starts = np.zeros((NCORES, NBL), np.int64)
    flat = counts.ravel().cumsum()
    starts.ravel()[1:] = flat[:-1]
    for p in range(NCORES):
        idx_t = np.zeros((P, neb), np.int32)
        sel_t = np.zeros((P, neb * P), np.float32)
        col = 0
        for b in range(NBL):
            s0, cnt = starts[p, b], counts[p, b]
            g = grow_s[s0:s0 + cnt]
            c = ncol_s[s0:s0 + cnt]
            ww = w_s[s0:s0 + cnt]
            for k in range(knb[b]):
                lo, hi = k * P, min((k + 1) * P, cnt)
                if hi > lo:
                    lanes = np.arange(hi - lo)
                    idx_t[lanes, col] = g[lo:hi]
                    sel_t[lanes, col * P + c[lo:hi]] = ww[lo:hi]
                col += 1
        idx_tabs.append(idx_t)
        sel_tabs.append(sel_t.astype(BFNP))
    return knb, idx_tabs, sel_tabs


def _pack_lhsT(xT, kg):
    """[kg*128, NPAD] -> [128, kg*NPAD] (partition-major kg blocks)."""
    return np.ascontiguousarray(
        xT.reshape(kg, P, NPAD).transpose(1, 0, 2).reshape(P, kg * NPAD))


def _pack_rhs(Wm, kg, n):
    """[kg*128, n] -> [128, kg*n]."""
    return np.ascontiguousarray(
        Wm.reshape(kg, P, n).transpose(1, 0, 2).reshape(P, kg * n))


# ------------------------------------------------------------- device build
def _build(knb, repeat=1):
    neb = sum(knb)
    nc = bass.Bass("TRN2", target_bir_lowering=False, debug=False,
                   num_devices=NCORES)

    def din(name, shape, dt):
        return nc.dram_tensor(name, shape, dt, kind="ExternalInput").ap()

    xT = din("xT", [P, KG1 * NPAD], BF)
    idx = din("idx", [P, neb], mybir.dt.int32)
    sel = din("sel", [P, neb * P], BF)
    w1 = din("w1", [P, KG1 * CH], BF)
    v1 = din("v1", [P, KG1 * CH], BF)
    wk = din("wk", [P, 6 * KGC * CH], BF)
    vk = din("vk", [P, 6 * KGC * CH], BF)
    wd = din("wd", [P, KGC * N_LABELS], BF)
    ball = din("ball", [P, NLAYERS * CH], F32)
    bdr = din("bdr", [P, N_LABELS], F32)
    out = nc.dram_tensor("out", [NPAD, N_LABELS], F32, kind="ExternalOutput").ap()

    with tile.TileContext(nc) as tc:
        with (
            tc.tile_pool(name="const", bufs=1) as cp,
            tc.tile_pool(name="ht", bufs=2) as htp,
            tc.tile_pool(name="mout", bufs=3) as mp,
            tc.tile_pool(name="msg", bufs=16) as msgp,
            tc.tile_pool(name="hact", bufs=2) as hp,
            tc.tile_pool(name="outs", bufs=2) as op,
            tc.tile_pool(name="psm", bufs=2, space="PSUM") as psm,
            tc.tile_pool(name="psagg", bufs=4, space="PSUM") as psagg,
            tc.tile_pool(name="pstr", bufs=2, space="PSUM") as pstr,
            tc.tile_pool(name="dram", bufs=1, space="DRAM") as dram,
        ):
            # ---- constants to SBUF
            xT_t = cp.tile([P, KG1 * NPAD], BF)
            nc.sync.dma_start(xT_t[:], xT[:])
            idx_t = cp.tile([P, neb], mybir.dt.int32)
            nc.sync.dma_start(idx_t[:], idx[:])
            sel_t = cp.tile([P, neb * P], BF)
            nc.sync.dma_start(sel_t[:], sel[:])
            w1_t = cp.tile([P, KG1 * CH], BF)
            nc.sync.dma_start(w1_t[:], w1[:])
            v1_t = cp.tile([P, KG1 * CH], BF)
            nc.sync.dma_start(v1_t[:], v1[:])
            wk_t = cp.tile([P, 6 * KGC * CH], BF)
            nc.sync.dma_start(wk_t[:], wk[:])
            vk_t = cp.tile([P, 6 * KGC * CH], BF)
            nc.sync.dma_start(vk_t[:], vk[:])
            wd_t = cp.tile([P, KGC * N_LABELS], BF)
            nc.sync.dma_start(wd_t[:], wd[:])
            ball_t = cp.tile([P, NLAYERS * CH], F32)
            nc.sync.dma_start(ball_t[:], ball[:])
            bdr_t = cp.tile([P, N_LABELS], F32)
            nc.sync.dma_start(bdr_t[:], bdr[:])
            ident = cp.tile([P, P], BF)
            make_identity(nc, ident[:])

            for rep in range(repeat):
                hT_cur = None
                for l in range(NLAYERS):
                    kg = KG1 if l == 0 else KGC
                    if l == 0:
                        lhsT_t, lw = xT_t, NPAD * KG1
                        wt = w1_t[:, :]
                        vt = v1_t[:, :]
                    else:
                        lhsT_t, lw = hT_cur, NPAD * KGC
                        wt = wk_t[:, (l - 1) * KGC * CH:l * KGC * CH]
                        vt = vk_t[:, (l - 1) * KGC * CH:l * KGC * CH]

                    # --- m = h @ W for own nodes; stage to DRAM for AllGather
                    ag_in = dram.tile([NPAD, CH], BF, tag="ag_in")
                    for b in range(NBL):
                        m_ps = psm.tile([P, CH], F32, tag="m")
                        for g in range(kg):
                            nc.tensor.matmul(
                                m_ps[:],
                                lhsT_t[:, g * NPAD + b * P:g * NPAD + (b + 1) * P],
                                wt[:, g * CH:(g + 1) * CH],
                                start=(g == 0), stop=(g == kg - 1))
                        m_bf = mp.tile([P, CH], BF, tag="mbf")
                        nc.vector.tensor_copy(m_bf[:], m_ps[:])
                        nc.sync.dma_start(ag_in[b * P:(b + 1) * P, :], m_bf[:])

                    ag_out = dram.tile([NCORES * NPAD, CH], BF,
                                       tag=f"ag_out{l}", addr_space="Shared")
                    nc.gpsimd.collective_compute(
                        "AllGather", mybir.AluOpType.bypass,
                        replica_groups=[list(range(NCORES))],
                        ins=[ag_in[:].opt()], outs=[ag_out[:].opt()])

                    # --- per node block: hV + scattered messages -> h'
                    hT_next = htp.tile([P, KGC * NPAD], BF, tag="hT")
                    col = 0
                    for b in range(NBL):
                        h_ps = psagg.tile([P, CH], F32, tag="agg")
                        for g in range(kg):
                            nc.tensor.matmul(
                                h_ps[:],
                                lhsT_t[:, g * NPAD + b * P:g * NPAD + (b + 1) * P],
                                vt[:, g * CH:(g + 1) * CH],
                                start=(g == 0), stop=False)
                        for k in range(knb[b]):
                            msg = msgp.tile([P, CH], BF, tag="msg")
                            nc.gpsimd.indirect_dma_start(
                                out=msg[:], out_offset=None,
                                in_=ag_out[:],
                                in_offset=bass.IndirectOffsetOnAxis(
                                    ap=idx_t[:, col:col + 1], axis=0))
                            nc.tensor.matmul(
                                h_ps[:],
                                sel_t[:, col * P:(col + 1) * P],
                                msg[:],
                                start=False, stop=(k == knb[b] - 1))
                            col += 1
                        nc.vector.tensor_add(
                            h_ps[:], h_ps[:], ball_t[:, l * CH:(l + 1) * CH])
                        h_bf = hp.tile([P, CH], BF, tag="h")
                        nc.scalar.activation(
                            h_bf[:], h_ps[:], mybir.ActivationFunctionType.Relu)
                        for cg in range(KGC):
                            tr_ps = pstr.tile([P, P], BF, tag="tr")
                            nc.tensor.transpose(
                                tr_ps[:], h_bf[:, cg * P:(cg + 1) * P], ident[:])
                            nc.vector.tensor_copy(
                                hT_next[:, cg * NPAD + b * P:cg * NPAD + (b + 1) * P],
                                tr_ps[:])
                    hT_cur = hT_next

                # ---- final dense: logits = h7 @ Wd + bd
                for b in range(NBL):
                    o_sb = op.tile([P, N_LABELS], F32, tag="o")
                    fps = []
                    for c in range(3):
                        fin_ps = psagg.tile([P, FIN_CHUNK], F32, tag="agg")
                        fps.append(fin_ps)
                    for g in range(KGC):
                        for c in range(3):
                            nc.tensor.matmul(
                                fps[c][:],
                                hT_cur[:, g * NPAD + b * P:g * NPAD + (b + 1) * P],
                                wd_t[:, g * N_LABELS + c * FIN_CHUNK:
                                     g * N_LABELS + (c + 1) * FIN_CHUNK],
                                start=(g == 0), stop=(g == KGC - 1))
                    for c in range(3):
                        sl = slice(c * FIN_CHUNK, (c + 1) * FIN_CHUNK)
                        nc.vector.tensor_add(fps[c][:], fps[c][:], bdr_t[:, sl])
                        nc.scalar.activation(
                            o_sb[:, sl], fps[c][:],
                            mybir.ActivationFunctionType.Copy)
                    if rep == repeat - 1:
                        nc.sync.dma_start(out[b * P:(b + 1) * P, :], o_sb[:])

    _split_excess_waits(nc)
    return nc


# ------------------------------------------------------------- entry point
def kernel(x, src, dst, W1, V1, b1, Wk, Vk, bk, Wd, bd, _repeat=1, _nc_cache={}):
    x = np.asarray(x, np.float32)
    knb, idx_tabs, sel_tabs = _prep_edges(src, dst)

    key = (tuple(knb), _repeat)
    if key not in _nc_cache:
        _nc_cache[key] = _build(knb, repeat=_repeat)
    nc = _nc_cache[key]

    # weights (replicated, host-packed)
    w1p = _pack_rhs(np.asarray(W1, np.float32), KG1, CH).astype(BFNP)
    v1p = _pack_rhs(np.asarray(V1, np.float32), KG1, CH).astype(BFNP)
    wkp = np.concatenate(
        [_pack_rhs(np.asarray(Wk[i], np.float32), KGC, CH) for i in range(6)],
        axis=1).astype(BFNP)
    vkp = np.concatenate(
        [_pack_rhs(np.asarray(Vk[i], np.float32), KGC, CH) for i in range(6)],
        axis=1).astype(BFNP)
    wdp = _pack_rhs(np.asarray(Wd, np.float32), KGC, N_LABELS).astype(BFNP)
    ballv = np.concatenate(
        [np.asarray(b1, np.float32)] + [np.asarray(bk[i], np.float32)
                                        for i in range(6)])
    ballp = np.broadcast_to(ballv, (P, NLAYERS * CH)).copy()
    bdp = np.broadcast_to(np.asarray(bd, np.float32), (P, N_LABELS)).copy()

    in_maps = []
    for p in range(NCORES):
        xp = np.zeros((NPAD, IN_F), np.float32)
        xp[:NPC] = x[p * NPC:(p + 1) * NPC]
        xTp = _pack_lhsT(np.ascontiguousarray(xp.T), KG1).astype(BFNP)
        in_maps.append({
            "xT": xTp, "idx": idx_tabs[p], "sel": sel_tabs[p],
            "w1": w1p, "v1": v1p, "wk": wkp, "vk": vkp, "wd": wdp,
            "ball": ballp, "bdr": bdp,
        })

    res = run_bass_kernel_spmd(nc, in_maps, core_ids=list(range(NCORES)))
    outp = np.empty((N_NODES, N_LABELS), np.float32)
    for p in range(NCORES):
        outp[p * NPC:(p + 1) * NPC] = res.results[p]["out"][:NPC]
    return outp



# revision 12
# speedup vs baseline: 1.6727x; 1.6727x over previous
"""ARMA-style GNN message passing on 8 TRN2 NeuronCores.

Reference computation (per layer, 7 layers):
    m   = h @ W                                  [N, CH]
    agg = segment_sum(w[:,None] * m[dst], src)   [N, CH]
    h'  = relu(agg + h @ V + b)
then logits = h @ Wd + bd.

Strategy (dst-partitioned edges + ReduceScatter):
  - 8 cores own 1250 nodes each (padded to 1280 = 10 blocks of 128).
  - Edge (s, d) is processed by the core owning d: the message row m[d]
    lives in that core's local m table, so gathers are local (no
    AllGather of m). Messages are scatter-added into per-src-block
    partial aggregates via PE matmuls with host-built bf16 "selection"
    matrices (segment-sum == sel.T @ msg_rows).
  - Host sorts each core's edges by global src block (80 blocks of 128
    padded-global rows); per (core, src block) the edge count is padded
    to a multiple of 128; the per-block count is the max over cores so
    the SPMD program is identical.
  - Per layer: m = h @ W for own nodes (PE) -> staged to local DRAM;
    batched dma_gather calls (CHUNK edge blocks each, ~1KB rows) pull
    per-edge message rows; per src block the sel matmuls accumulate
    into PSUM; partials staged to DRAM; one ReduceScatter sums the
    [80*128, CH] partials and hands each core its own 10 blocks.
    h@V + bias runs on PE during the ReduceScatter; epilogue adds
    agg + hV, applies relu, and PE-transposes into the next layer's
    stationary operand hT.
  - Final dense layer per core; host concatenates.

All matmuls run in bf16 with fp32 PSUM accumulation.
"""
import numpy as np
import ml_dtypes

import concourse.bass as bass
import concourse.tile as tile
import concourse.mybir as mybir
from concourse.vector_clock import ScopedClock
from concourse.bass_utils import run_bass_kernel_spmd
from concourse.masks import make_identity
from concourse import library_config
from concourse.library_overlay import lower_extended_insts

# ---------------------------------------------------------------- constants
N_NODES = 10000
N_EDGES = 160000
IN_F = 256
CH = 512
N_LABELS = 1440
NCORES = 8
NPC = N_NODES // NCORES      # 1250 nodes per core
P = 128
NBL = 10                     # node blocks per core (10*128 = 1280)
NPAD = NBL * P               # padded nodes per core
NSB = NCORES * NBL           # 80 global src blocks
NLAYERS = 7
KG1 = IN_F // P              # 2 contraction blocks in layer 1
KGC = CH // P                # 4 contraction blocks in layers 2..7
FIN_CHUNK = 480              # 1440 = 3 * 480, fits one PSUM bank in f32
CHUNK = 16                   # edge blocks per dma_gather call

BF = mybir.dt.bfloat16
F32 = mybir.dt.float32
BFNP = ml_dtypes.bfloat16


# ------------------------------------------------------- walrus workarounds
def _patched_drain_and_barrier(self, tick_clock, wait_clock):
    # This walrus build rejects >1-2 sync waits on one TPB_CTRL; put the
    # kernel-tail drain's waits on separate preceding SP nops instead.
    nc = self.nc
    probe = nc.sync.nop(nofuse=True, hint="drain_waits")
    wait_clock.add_sem_waits(probe.ins, ScopedClock({None: tick_clock.global_clock}))
    si = probe.ins.sync_info
    waits = list(si.on_wait) if si is not None else []
    if len(waits) > 1:
        si.on_wait = waits[:1]
        for i in range(1, len(waits)):
            n2 = nc.sync.nop(nofuse=True, hint=f"drain_waits_{i}")
            n2.ins.sync_info = mybir.SyncInfo(on_wait=[waits[i]], on_update=[])
    nc.sync.drain()
    nc.all_engine_barrier()
    assert self.sems is not None
    popped = nc._tile_sem_poison_stack.pop()
    assert popped is self._sem_poison
    nc.clear_and_free_semaphores(list(self.sems.allocated().values()))
    nc.all_engine_barrier()


tile.TileContext._drain_and_barrier = _patched_drain_and_barrier


def _split_excess_waits(nc, limit=1):
    # Same ISA restriction for ordinary instructions: hoist excess sync
    # waits onto injected same-engine nops placed just before.
    for func in nc.m.functions:
        for bb in func.blocks:
            out = []
            for ins in bb.instructions:
                si = ins.sync_info
                if si is not None and si.on_wait and len(si.on_wait) > limit:
                    waits = list(si.on_wait)
                    excess, keep = waits[:-limit], waits[-limit:]
                    for i in range(0, len(excess), limit):
                        out.append(mybir.InstNoOp(
                            name=f"{ins.name}_xw{i}",
                            engine=ins.engine,
                            ins=[], outs=[],
                            sync_info=mybir.SyncInfo(
                                on_wait=excess[i:i + limit], on_update=[]),
                        ))
                    si.on_wait = keep
                out.append(ins)
            bb.instructions[:] = out


# ------------------------------------------------------------- host prep
def _prep_edges(src, dst):
    """Partition edges by owning core of dst; per core group edges by
    global src block. Returns (kgrp, idx_tabs, sel_tabs):
      kgrp[sb]   edge-block count of src block sb (same on all cores)
      idx_tabs[p] int16 [128, NEB*8] 16-wrapped local m-row gather indices
      sel_tabs[p] bf16 [128, NEB*128] selection/weight matrices
    """
    src = np.asarray(src).astype(np.int64)
    dst = np.asarray(dst).astype(np.int64)
    deg_out = np.maximum(np.bincount(src, minlength=N_NODES), 1.0).astype(np.float32)
    deg_in = np.maximum(np.bincount(dst, minlength=N_NODES), 1.0).astype(np.float32)
    w = (1.0 / np.sqrt(deg_out[src] * deg_in[dst])).astype(np.float32)

    core = dst // NPC                       # owning core (dst side)
    lrow = (dst % NPC).astype(np.int64)     # gather row in local m table
    g_src = (src // NPC) * NPAD + (src % NPC)
    sb = g_src // P                         # global src block 0..79
    scol = g_src % P                        # column within sel matrix

    order = np.lexsort((sb, core))
    core_s, sb_s = core[order], sb[order]
    lrow_s, scol_s, w_s = lrow[order], scol[order], w[order]
    counts = np.zeros((NCORES, NSB), np.int64)
    np.add.at(counts, (core_s, sb_s), 1)
    kgrp = [max(1, int(-(-counts[:, b].max() // P))) for b in range(NSB)]
    neb = sum(kgrp)

    idx_tabs, sel_tabs = [], []
    starts = np.zeros((NCORES, NSB), np.int64)
    flat = counts.ravel().cumsum()
    starts.ravel()[1:] = flat[:-1]
    for p in range(NCORES):
        flat_idx = np.zeros(neb * P, np.int64)      # slot -> local m row
        sel_t = np.zeros((P, neb * P), np.float32)
        col = 0
        for b in range(NSB):
            s0, cnt = starts[p, b], counts[p, b]
            g = lrow_s[s0:s0 + cnt]
            c = scol_s[s0:s0 + cnt]
            ww = w_s[s0:s0 + cnt]
            for k in range(kgrp[b]):
                lo, hi = k * P, min((k + 1) * P, cnt)
                if hi > lo:
                    lanes = np.arange(hi - lo)
                    flat_idx[col * P + lanes] = g[lo:hi]
                    sel_t[lanes, col * P + c[lo:hi]] = ww[lo:hi]
                col += 1
        # 16-wrap per gather call of CHUNK blocks: slot i of call c lands
        # at [i%16, c*CHUNK*8 + i//16], replicated across the 8 Q7 cores'
        # partition groups (each Q7 core reads its own 16 partitions)
        idx_t = np.zeros((P, neb * 8), np.int16)
        gidx = np.arange(neb * P)
        call = gidx // (CHUNK * P)
        i_in = gidx % (CHUNK * P)
        for q7 in range(8):
            idx_t[16 * q7 + i_in % 16, call * CHUNK * 8 + i_in // 16] = flat_idx
        idx_tabs.append(idx_t)
        sel_tabs.append(sel_t.astype(BFNP))
    return kgrp, idx_tabs, sel_tabs


def _pack_lhsT(xT, kg):
    """[kg*128, NPAD] -> [128, kg*NPAD] (partition-major kg blocks)."""
    return np.ascontiguousarray(
        xT.reshape(kg, P, NPAD).transpose(1, 0, 2).reshape(P, kg * NPAD))


def _pack_rhs(Wm, kg, n):
    """[kg*128, n] -> [128, kg*n]."""
    return np.ascontiguousarray(
        Wm.reshape(kg, P, n).transpose(1, 0, 2).reshape(P, kg * n))


# ------------------------------------------------------------- device build
def _build(kgrp, repeat=1):
    neb = sum(kgrp)
    ncalls = -(-neb // CHUNK)
    nc = bass.Bass("TRN2", target_bir_lowering=False, debug=False,
                   num_devices=NCORES)

    def din(name, shape, dt):
        return nc.dram_tensor(name, shape, dt, kind="ExternalInput").ap()

    xT = din("xT", [P, KG1 * NPAD], BF)
    idx = din("idx", [P, neb * 8], mybir.dt.int16)
    sel = din("sel", [P, neb * P], BF)
    w1 = din("w1", [P, KG1 * CH], BF)
    v1 = din("v1", [P, KG1 * CH], BF)
    wk = din("wk", [P, 6 * KGC * CH], BF)
    vk = din("vk", [P, 6 * KGC * CH], BF)
    wd = din("wd", [P, KGC * N_LABELS], BF)
    ball = din("ball", [P, NLAYERS * CH], F32)
    bdr = din("bdr", [P, N_LABELS], F32)
    out = nc.dram_tensor("out", [NPAD, N_LABELS], F32, kind="ExternalOutput").ap()

    with tile.TileContext(nc) as tc:
        with (
            tc.tile_pool(name="const", bufs=1) as cp,
            tc.tile_pool(name="wv", bufs=2) as wvp,
            tc.tile_pool(name="ht", bufs=2) as htp,
            tc.tile_pool(name="mout", bufs=3) as mp,
            tc.tile_pool(name="msg", bufs=2) as msgp,
            tc.tile_pool(name="part", bufs=3) as prp,
            tc.tile_pool(name="hvs", bufs=2) as hvp,
            tc.tile_pool(name="aggin", bufs=2) as aggp,
            tc.tile_pool(name="hact", bufs=2) as hp,
            tc.tile_pool(name="outs", bufs=1) as op,
            tc.tile_pool(name="psm", bufs=2, space="PSUM") as psm,
            tc.tile_pool(name="psagg", bufs=3, space="PSUM") as psagg,
            tc.tile_pool(name="pshv", bufs=1, space="PSUM") as pshv,
            tc.tile_pool(name="pstr", bufs=2, space="PSUM") as pstr,
            tc.tile_pool(name="dram", bufs=1, space="DRAM") as dram,
        ):
            nc.gpsimd.load_library(library_config.mlp)
            # ---- constants to SBUF
            xT_t = cp.tile([P, KG1 * NPAD], BF)
            nc.sync.dma_start(xT_t[:], xT[:])
            idx_t = cp.tile([P, neb * 8], mybir.dt.int16)
            nc.sync.dma_start(idx_t[:], idx[:])
            sel_t = cp.tile([P, neb * P], BF)
            nc.sync.dma_start(sel_t[:], sel[:])
            w1_t = cp.tile([P, KG1 * CH], BF)
            nc.sync.dma_start(w1_t[:], w1[:])
            v1_t = cp.tile([P, KG1 * CH], BF)
            nc.sync.dma_start(v1_t[:], v1[:])
            wd_t = cp.tile([P, KGC * N_LABELS], BF)
            nc.sync.dma_start(wd_t[:], wd[:])
            ball_t = cp.tile([P, NLAYERS * CH], F32)
            nc.sync.dma_start(ball_t[:], ball[:])
            bdr_t = cp.tile([P, N_LABELS], F32)
            nc.sync.dma_start(bdr_t[:], bdr[:])
            ident = cp.tile([P, P], BF)
            make_identity(nc, ident[:])

            # num_idxs registers for the batched gathers (to_reg on an int
            # immediate has no free-register pool under TileContext)
            rem = neb % CHUNK
            nir_full = nc.gpsimd.alloc_register("nir_full")
            nc.gpsimd.reg_mov(nir_full, CHUNK * P)
            nir_rem = None
            if rem:
                nir_rem = nc.gpsimd.alloc_register("nir_rem")
                nc.gpsimd.reg_mov(nir_rem, rem * P)

            for rep in range(repeat):
                hT_cur = None
                for l in range(NLAYERS):
                    kg = KG1 if l == 0 else KGC
                    if l == 0:
                        lhsT_t = xT_t
                        wt = w1_t[:, :]
                        vt = v1_t[:, :]
                    else:
                        lhsT_t = hT_cur
                        wt_t = wvp.tile([P, KGC * CH], BF, tag="wk")
                        nc.sync.dma_start(
                            wt_t[:], wk[:, (l - 1) * KGC * CH:l * KGC * CH])
                        vt_t = wvp.tile([P, KGC * CH], BF, tag="vk")
                        nc.sync.dma_start(
                            vt_t[:], vk[:, (l - 1) * KGC * CH:l * KGC * CH])
                        wt = wt_t[:, :]
                        vt = vt_t[:, :]

                    # --- m = h @ W for own nodes; stage to local DRAM
                    m_dram = dram.tile([NPAD, CH], BF, tag=f"m{l}")
                    for b in range(NBL):
                        m_ps = psm.tile([P, CH], F32, tag="m")
                        for g in range(kg):
                            nc.tensor.matmul(
                                m_ps[:],
                                lhsT_t[:, g * NPAD + b * P:g * NPAD + (b + 1) * P],
                                wt[:, g * CH:(g + 1) * CH],
                                start=(g == 0), stop=(g == kg - 1))
                        m_bf = mp.tile([P, CH], BF, tag="mbf")
                        nc.vector.tensor_copy(m_bf[:], m_ps[:])
                        nc.sync.dma_start(m_dram[b * P:(b + 1) * P, :], m_bf[:])

                    # --- gathers (batched) + scatter matmuls, interleaved
                    rs_in = dram.tile([NSB * P, CH], BF, tag=f"ri{l}")
                    msg_tiles = [None] * ncalls
                    col = 0
                    for b in range(NSB):
                        h_ps = psagg.tile([P, CH], F32, tag="agg")
                        for k in range(kgrp[b]):
                            ci, wi = col // CHUNK, col % CHUNK
                            if wi == 0:
                                nblk = min(CHUNK, neb - ci * CHUNK)
                                mt = msgp.tile([P, CHUNK * CH], BF, tag="msg")
                                out3 = mt[:].rearrange(
                                    "p (k c) -> p k c", c=CH)[:, :nblk, :]
                                nc.gpsimd.dma_gather(
                                    out_ap=out3,
                                    in_ap=m_dram[:, :],
                                    idxs_ap=idx_t[:, ci * CHUNK * 8:
                                                  ci * CHUNK * 8 + nblk * 8],
                                    num_idxs=nblk * P,
                                    num_idxs_reg=(nir_full if nblk == CHUNK
                                                  else nir_rem),
                                    elem_size=CH, single_packet=False)
                                msg_tiles[ci] = mt
                            mt = msg_tiles[ci]
                            nc.tensor.matmul(
                                h_ps[:],
                                sel_t[:, col * P:(col + 1) * P],
                                mt[:, wi * CH:(wi + 1) * CH],
                                start=(k == 0), stop=(k == kgrp[b] - 1))
                            col += 1
                        pr_bf = prp.tile([P, CH], BF, tag="pr")
                        nc.vector.tensor_copy(pr_bf[:], h_ps[:])
                        nc.sync.dma_start(rs_in[b * P:(b + 1) * P, :], pr_bf[:])

                    # --- ReduceScatter: sum partials, each core keeps its rows
                    rs_out = dram.tile([NPAD, CH], BF, tag=f"ro{l}")
                    nc.gpsimd.collective_compute(
                        "ReduceScatter", mybir.AluOpType.add,
                        replica_groups=[list(range(NCORES))],
                        ins=[rs_in[:].opt()], outs=[rs_out[:].opt()])

                    # --- hV + bias for own nodes (overlaps the collective)
                    hv_sb = hvp.tile([P, NBL * CH], BF, tag="hv")
                    for b in range(NBL):
                        v_ps = pshv.tile([P, CH], F32, tag="hv")
                        for g in range(kg):
                            nc.tensor.matmul(
                                v_ps[:],
                                lhsT_t[:, g * NPAD + b * P:g * NPAD + (b + 1) * P],
                                vt[:, g * CH:(g + 1) * CH],
                                start=(g == 0), stop=(g == kg - 1))
                        nc.vector.tensor_add(
                            v_ps[:], v_ps[:], ball_t[:, l * CH:(l + 1) * CH])
                        nc.vector.tensor_copy(hv_sb[:, b * CH:(b + 1) * CH],
                                              v_ps[:])

                    # --- epilogue: h' = relu(agg + hV + b); transpose to hT
                    hT_next = htp.tile([P, KGC * NPAD], BF, tag="hT")
                    for b in range(NBL):
                        agg_t = aggp.tile([P, CH], BF, tag="agg_in")
                        nc.sync.dma_start(agg_t[:], rs_out[b * P:(b + 1) * P, :])
                        h_bf = hp.tile([P, CH], BF, tag="h")
                        nc.vector.tensor_add(
                            h_bf[:], agg_t[:], hv_sb[:, b * CH:(b + 1) * CH])
                        nc.scalar.activation(
                            h_bf[:], h_bf[:], mybir.ActivationFunctionType.Relu)
                        for cg in range(KGC):
                            tr_ps = pstr.tile([P, P], BF, tag="tr")
                            nc.tensor.transpose(
                                tr_ps[:], h_bf[:, cg * P:(cg + 1) * P], ident[:])
                            nc.vector.tensor_copy(
                                hT_next[:, cg * NPAD + b * P:cg * NPAD + (b + 1) * P],
                                tr_ps[:])
                    hT_cur = hT_next

                # ---- final dense: logits = h7 @ Wd + bd
                for b in range(NBL):
                    o_sb = op.tile([P, N_LABELS], F32, tag="o")
                    fps = []
                    for c in range(3):
                        fin_ps = psagg.tile([P, FIN_CHUNK], F32, tag="agg")
                        fps.append(fin_ps)
                    for g in range(KGC):
                        for c in range(3):
                            nc.tensor.matmul(
                                fps[c][:],
                                hT_cur[:, g * NPAD + b * P:g * NPAD + (b + 1) * P],
                                wd_t[:, g * N_LABELS + c * FIN_CHUNK:
                                     g * N_LABELS + (c + 1) * FIN_CHUNK],
                                start=(g == 0), stop=(g == KGC - 1))
                    for c in range(3):
                        sl = slice(c * FIN_CHUNK, (c + 1) * FIN_CHUNK)
                        nc.vector.tensor_add(fps[c][:], fps[c][:], bdr_t[:, sl])
                        nc.scalar.activation(
                            o_sb[:, sl], fps[c][:],
                            mybir.ActivationFunctionType.Copy)
                    if rep == repeat - 1:
                        nc.sync.dma_start(out[b * P:(b + 1) * P, :], o_sb[:])

    _split_excess_waits(nc)
    lower_extended_insts(nc)
    return nc


# ------------------------------------------------------------- entry point
def kernel(x, src, dst, W1, V1, b1, Wk, Vk, bk, Wd, bd, _repeat=1, _nc_cache={}):
    x = np.asarray(x, np.float32)
    kgrp, idx_tabs, sel_tabs = _prep_edges(src, dst)

    key = (tuple(kgrp), _repeat)
    if key not in _nc_cache:
        _nc_cache[key] = _build(kgrp, repeat=_repeat)
    nc = _nc_cache[key]

    # weights (replicated, host-packed)
    w1p = _pack_rhs(np.asarray(W1, np.float32), KG1, CH).astype(BFNP)
    v1p = _pack_rhs(np.asarray(V1, np.float32), KG1, CH).astype(BFNP)
    wkp = np.concatenate(
        [_pack_rhs(np.asarray(Wk[i], np.float32), KGC, CH) for i in range(6)],
        axis=1).astype(BFNP)
    vkp = np.concatenate(
        [_pack_rhs(np.asarray(Vk[i], np.float32), KGC, CH) for i in range(6)],
        axis=1).astype(BFNP)
    wdp = _pack_rhs(np.asarray(Wd, np.float32), KGC, N_LABELS).astype(BFNP)
    ballv = np.concatenate(
        [np.asarray(b1, np.float32)] + [np.asarray(bk[i], np.float32)
                                        for i in range(6)])
    ballp = np.broadcast_to(ballv, (P, NLAYERS * CH)).copy()
    bdp = np.broadcast_to(np.asarray(bd, np.float32), (P, N_LABELS)).copy()

    in_maps = []
    for p in range(NCORES):
        xp = np.zeros((NPAD, IN_F), np.float32)
        xp[:NPC] = x[p * NPC:(p + 1) * NPC]
        xTp = _pack_lhsT(np.ascontiguousarray(xp.T), KG1).astype(BFNP)
        in_maps.append({
            "xT": xTp, "idx": idx_tabs[p], "sel": sel_tabs[p],
            "w1": w1p, "v1": v1p, "wk": wkp, "vk": vkp, "wd": wdp,
            "ball": ballp, "bdr": bdp,
        })

    res = run_bass_kernel_spmd(nc, in_maps, core_ids=list(range(NCORES)))
    outp = np.empty((N_NODES, N_LABELS), np.float32)
    for p in range(NCORES):
        outp[p * NPC:(p + 1) * NPC] = res.results[p]["out"][:NPC]
    return outp


# revision 18
# speedup vs baseline: 2.2364x; 1.3370x over previous
"""ARMA-style GNN message passing on 8 TRN2 NeuronCores.

Reference computation (per layer, 7 layers):
    m   = h @ W                                  [N, CH]
    agg = segment_sum(w[:,None] * m[dst], src)   [N, CH]
    h'  = relu(agg + h @ V + b)
then logits = h @ Wd + bd.

Strategy (dst-partitioned edges + ReduceScatter):
  - 8 cores own 1250 nodes each (padded to 1280 = 10 blocks of 128).
  - Edge (s, d) is processed by the core owning d: the message row m[d]
    lives in that core's local m table, so gathers are local (no
    AllGather of m). Messages are scatter-added into per-src-block
    partial aggregates via PE matmuls with host-built bf16 "selection"
    matrices (segment-sum == sel.T @ msg_rows).
  - Host sorts each core's edges by global src block (80 blocks of 128
    padded-global rows); per (core, src block) the edge count is padded
    to a multiple of 128; the per-block count is the max over cores so
    the SPMD program is identical.
  - Per layer: m = h @ W for own nodes (PE) -> staged to local DRAM;
    batched dma_gather calls (CHUNK edge blocks each, ~1KB rows) pull
    per-edge message rows; per src block the sel matmuls accumulate
    into PSUM; partials staged to DRAM; one ReduceScatter sums the
    [80*128, CH] partials and hands each core its own 10 blocks.
    h@V + bias runs on PE during the ReduceScatter; epilogue adds
    agg + hV, applies relu, and PE-transposes into the next layer's
    stationary operand hT.
  - Final dense layer per core; host concatenates.

All matmuls run in bf16 with fp32 PSUM accumulation.
"""
import numpy as np
import ml_dtypes

import concourse.bass as bass
import concourse.tile as tile
import concourse.mybir as mybir
from concourse.vector_clock import ScopedClock
from concourse.bass_utils import run_bass_kernel_spmd
from concourse.masks import make_identity
from concourse import library_config
from concourse.library_overlay import lower_extended_insts

# ---------------------------------------------------------------- constants
N_NODES = 10000
N_EDGES = 160000
IN_F = 256
CH = 512
N_LABELS = 1440
NCORES = 8
NPC = N_NODES // NCORES      # 1250 nodes per core
P = 128
NBL = 10                     # node blocks per core (10*128 = 1280)
NPAD = NBL * P               # padded nodes per core
NSB = NCORES * NBL           # 80 global src blocks
NLAYERS = 7
KG1 = IN_F // P              # 2 contraction blocks in layer 1
KGC = CH // P                # 4 contraction blocks in layers 2..7
FIN_CHUNK = 480              # 1440 = 3 * 480, fits one PSUM bank in f32
CHUNK = 8                    # edge blocks per dma_gather call

BF = mybir.dt.bfloat16
F8 = mybir.dt.float8e4          # e4m3: message table / gather payload
F32 = mybir.dt.float32
BFNP = ml_dtypes.bfloat16


# ------------------------------------------------------- walrus workarounds
def _patched_drain_and_barrier(self, tick_clock, wait_clock):
    # This walrus build rejects >1-2 sync waits on one TPB_CTRL; put the
    # kernel-tail drain's waits on separate preceding SP nops instead.
    nc = self.nc
    probe = nc.sync.nop(nofuse=True, hint="drain_waits")
    wait_clock.add_sem_waits(probe.ins, ScopedClock({None: tick_clock.global_clock}))
    si = probe.ins.sync_info
    waits = list(si.on_wait) if si is not None else []
    if len(waits) > 1:
        si.on_wait = waits[:1]
        for i in range(1, len(waits)):
            n2 = nc.sync.nop(nofuse=True, hint=f"drain_waits_{i}")
            n2.ins.sync_info = mybir.SyncInfo(on_wait=[waits[i]], on_update=[])
    nc.sync.drain()
    nc.all_engine_barrier()
    assert self.sems is not None
    popped = nc._tile_sem_poison_stack.pop()
    assert popped is self._sem_poison
    nc.clear_and_free_semaphores(list(self.sems.allocated().values()))
    nc.all_engine_barrier()


tile.TileContext._drain_and_barrier = _patched_drain_and_barrier


def _split_excess_waits(nc, limit=1):
    # Same ISA restriction for ordinary instructions: hoist excess sync
    # waits onto injected same-engine nops placed just before.
    for func in nc.m.functions:
        for bb in func.blocks:
            out = []
            for ins in bb.instructions:
                si = ins.sync_info
                if si is not None and si.on_wait and len(si.on_wait) > limit:
                    waits = list(si.on_wait)
                    excess, keep = waits[:-limit], waits[-limit:]
                    for i in range(0, len(excess), limit):
                        out.append(mybir.InstNoOp(
                            name=f"{ins.name}_xw{i}",
                            engine=ins.engine,
                            ins=[], outs=[],
                            sync_info=mybir.SyncInfo(
                                on_wait=excess[i:i + limit], on_update=[]),
                        ))
                    si.on_wait = keep
                out.append(ins)
            bb.instructions[:] = out


# ------------------------------------------------------------- host prep
def _prep_edges(src, dst):
    """Partition edges by owning core of dst; per core group edges by
    global src block. Returns (kgrp, idx_tabs, sel_tabs):
      kgrp[sb]   edge-block count of src block sb (same on all cores)
      idx_tabs[p] int16 [128, NEB*8] 16-wrapped local m-row gather indices
      sel_tabs[p] bf16 [128, NEB*128] selection/weight matrices
    """
    src = np.asarray(src).astype(np.int64)
    dst = np.asarray(dst).astype(np.int64)
    deg_out = np.maximum(np.bincount(src, minlength=N_NODES), 1.0).astype(np.float32)
    deg_in = np.maximum(np.bincount(dst, minlength=N_NODES), 1.0).astype(np.float32)
    w = (1.0 / np.sqrt(deg_out[src] * deg_in[dst])).astype(np.float32)

    core = dst // NPC                       # owning core (dst side)
    lrow = (dst % NPC).astype(np.int64)     # gather row in local m table
    g_src = (src // NPC) * NPAD + (src % NPC)
    sb = g_src // P                         # global src block 0..79
    scol = g_src % P                        # column within sel matrix

    order = np.lexsort((sb, core))
    core_s, sb_s = core[order], sb[order]
    lrow_s, scol_s, w_s = lrow[order], scol[order], w[order]
    counts = np.zeros((NCORES, NSB), np.int64)
    np.add.at(counts, (core_s, sb_s), 1)
    kgrp = [max(1, int(-(-counts[:, b].max() // P))) for b in range(NSB)]
    neb = sum(kgrp)

    idx_tabs, sel_tabs = [], []
    starts = np.zeros((NCORES, NSB), np.int64)
    flat = counts.ravel().cumsum()
    starts.ravel()[1:] = flat[:-1]
    for p in range(NCORES):
        flat_idx = np.zeros(neb * P, np.int64)      # slot -> local m row
        sel_t = np.zeros((P, neb * P), np.float32)
        col = 0
        for b in range(NSB):
            s0, cnt = starts[p, b], counts[p, b]
            g = lrow_s[s0:s0 + cnt]
            c = scol_s[s0:s0 + cnt]
            ww = w_s[s0:s0 + cnt]
            for k in range(kgrp[b]):
                lo, hi = k * P, min((k + 1) * P, cnt)
                if hi > lo:
                    lanes = np.arange(hi - lo)
                    flat_idx[col * P + lanes] = g[lo:hi]
                    sel_t[lanes, col * P + c[lo:hi]] = ww[lo:hi]
                col += 1
        # 16-wrap per gather call of CHUNK blocks: slot i of call c lands
        # at [i%16, c*CHUNK*8 + i//16], replicated across the 8 Q7 cores'
        # partition groups (each Q7 core reads its own 16 partitions)
        idx_t = np.zeros((P, neb * 8), np.int16)
        gidx = np.arange(neb * P)
        call = gidx // (CHUNK * P)
        i_in = gidx % (CHUNK * P)
        for q7 in range(8):
            idx_t[16 * q7 + i_in % 16, call * CHUNK * 8 + i_in // 16] = flat_idx
        idx_tabs.append(idx_t)
        sel_tabs.append(sel_t.astype(BFNP))
    return kgrp, idx_tabs, sel_tabs


def _pack_lhsT(xT, kg):
    """[kg*128, NPAD] -> [128, kg*NPAD] (partition-major kg blocks)."""
    return np.ascontiguousarray(
        xT.reshape(kg, P, NPAD).transpose(1, 0, 2).reshape(P, kg * NPAD))


def _pack_rhs(Wm, kg, n):
    """[kg*128, n] -> [128, kg*n]."""
    return np.ascontiguousarray(
        Wm.reshape(kg, P, n).transpose(1, 0, 2).reshape(P, kg * n))


# ------------------------------------------------------------- device build
def _build(kgrp, repeat=1):
    neb = sum(kgrp)
    ncalls = -(-neb // CHUNK)
    nc = bass.Bass("TRN2", target_bir_lowering=False, debug=False,
                   num_devices=NCORES)

    def din(name, shape, dt):
        return nc.dram_tensor(name, shape, dt, kind="ExternalInput").ap()

    xT = din("xT", [P, KG1 * NPAD], BF)
    idx = din("idx", [P, neb * 8], mybir.dt.int16)
    sel = din("sel", [P, neb * P], BF)
    w1 = din("w1", [P, KG1 * CH], BF)
    v1 = din("v1", [P, KG1 * CH], BF)
    wk = din("wk", [P, 6 * KGC * CH], BF)
    vk = din("vk", [P, 6 * KGC * CH], BF)
    wd = din("wd", [P, KGC * N_LABELS], BF)
    ball = din("ball", [P, NLAYERS * CH], F32)
    bdr = din("bdr", [P, N_LABELS], F32)
    out = nc.dram_tensor("out", [NPAD, N_LABELS], F32, kind="ExternalOutput").ap()

    with tile.TileContext(nc) as tc:
        with (
            tc.tile_pool(name="const", bufs=1) as cp,
            tc.tile_pool(name="wv", bufs=2) as wvp,
            tc.tile_pool(name="ht", bufs=2) as htp,
            tc.tile_pool(name="mout", bufs=3) as mp,
            tc.tile_pool(name="msg", bufs=4) as msgp,
            tc.tile_pool(name="part", bufs=3) as prp,
            tc.tile_pool(name="hvs", bufs=2) as hvp,
            tc.tile_pool(name="aggin", bufs=2) as aggp,
            tc.tile_pool(name="hact", bufs=2) as hp,
            tc.tile_pool(name="outs", bufs=1) as op,
            tc.tile_pool(name="psm", bufs=2, space="PSUM") as psm,
            tc.tile_pool(name="psagg", bufs=3, space="PSUM") as psagg,
            tc.tile_pool(name="pshv", bufs=1, space="PSUM") as pshv,
            tc.tile_pool(name="pstr", bufs=2, space="PSUM") as pstr,
            tc.tile_pool(name="dram", bufs=1, space="DRAM") as dram,
        ):
            nc.gpsimd.load_library(library_config.mlp)
            # ---- constants to SBUF
            xT_t = cp.tile([P, KG1 * NPAD], BF)
            nc.sync.dma_start(xT_t[:], xT[:])
            idx_t = cp.tile([P, neb * 8], mybir.dt.int16)
            nc.sync.dma_start(idx_t[:], idx[:])
            sel_t = cp.tile([P, neb * P], BF)
            nc.sync.dma_start(sel_t[:], sel[:])
            w1_t = cp.tile([P, KG1 * CH], BF)
            nc.sync.dma_start(w1_t[:], w1[:])
            v1_t = cp.tile([P, KG1 * CH], BF)
            nc.sync.dma_start(v1_t[:], v1[:])
            wd_t = cp.tile([P, KGC * N_LABELS], BF)
            nc.sync.dma_start(wd_t[:], wd[:])
            ball_t = cp.tile([P, NLAYERS * CH], F32)
            nc.sync.dma_start(ball_t[:], ball[:])
            bdr_t = cp.tile([P, N_LABELS], F32)
            nc.sync.dma_start(bdr_t[:], bdr[:])
            ident = cp.tile([P, P], BF)
            make_identity(nc, ident[:])

            # num_idxs registers for the batched gathers (to_reg on an int
            # immediate has no free-register pool under TileContext)
            rem = neb % CHUNK
            nir_full = nc.gpsimd.alloc_register("nir_full")
            nc.gpsimd.reg_mov(nir_full, CHUNK * P)
            nir_rem = None
            if rem:
                nir_rem = nc.gpsimd.alloc_register("nir_rem")
                nc.gpsimd.reg_mov(nir_rem, rem * P)

            for rep in range(repeat):
                hT_cur = None
                for l in range(NLAYERS):
                    kg = KG1 if l == 0 else KGC
                    if l == 0:
                        lhsT_t = xT_t
                        wt = w1_t[:, :]
                        vt = v1_t[:, :]
                    else:
                        lhsT_t = hT_cur
                        wt_t = wvp.tile([P, KGC * CH], BF, tag="wk")
                        nc.sync.dma_start(
                            wt_t[:], wk[:, (l - 1) * KGC * CH:l * KGC * CH])
                        vt_t = wvp.tile([P, KGC * CH], BF, tag="vk")
                        nc.sync.dma_start(
                            vt_t[:], vk[:, (l - 1) * KGC * CH:l * KGC * CH])
                        wt = wt_t[:, :]
                        vt = vt_t[:, :]

                    # --- m = h @ W for own nodes; stage to local DRAM (fp8)
                    m_dram = dram.tile([NPAD, CH], F8, tag=f"m{l}")
                    for b in range(NBL):
                        m_ps = psm.tile([P, CH], F32, tag="m")
                        for g in range(kg):
                            nc.tensor.matmul(
                                m_ps[:],
                                lhsT_t[:, g * NPAD + b * P:g * NPAD + (b + 1) * P],
                                wt[:, g * CH:(g + 1) * CH],
                                start=(g == 0), stop=(g == kg - 1))
                        m_bf = mp.tile([P, CH], F8, tag="mbf")
                        nc.vector.tensor_copy(m_bf[:], m_ps[:])
                        nc.sync.dma_start(m_dram[b * P:(b + 1) * P, :], m_bf[:])

                    # --- gathers (batched) + scatter matmuls, interleaved
                    rs_in = dram.tile([NSB * P, CH], BF, tag=f"ri{l}")
                    msg_tiles = [None] * ncalls
                    col = 0
                    for b in range(NSB):
                        h_ps = psagg.tile([P, CH], F32, tag="agg")
                        for k in range(kgrp[b]):
                            ci, wi = col // CHUNK, col % CHUNK
                            if wi == 0:
                                nblk = min(CHUNK, neb - ci * CHUNK)
                                mt = msgp.tile([P, CHUNK * CH], F8, tag="msg")
                                out3 = mt[:].rearrange(
                                    "p (k c) -> p k c", c=CH)[:, :nblk, :]
                                nc.gpsimd.dma_gather(
                                    out_ap=out3,
                                    in_ap=m_dram[:, :],
                                    idxs_ap=idx_t[:, ci * CHUNK * 8:
                                                  ci * CHUNK * 8 + nblk * 8],
                                    num_idxs=nblk * P,
                                    num_idxs_reg=(nir_full if nblk == CHUNK
                                                  else nir_rem),
                                    elem_size=CH, single_packet=False)
                                msg_tiles[ci] = mt
                            mt = msg_tiles[ci]
                            nc.tensor.matmul(
                                h_ps[:],
                                sel_t[:, col * P:(col + 1) * P],
                                mt[:, wi * CH:(wi + 1) * CH],
                                start=(k == 0), stop=(k == kgrp[b] - 1))
                            col += 1
                        pr_bf = prp.tile([P, CH], BF, tag="pr")
                        nc.scalar.activation(
                            pr_bf[:], h_ps[:],
                            mybir.ActivationFunctionType.Copy)
                        nc.sync.dma_start(rs_in[b * P:(b + 1) * P, :], pr_bf[:])

                    # --- ReduceScatter: sum partials, each core keeps its rows
                    rs_out = dram.tile([NPAD, CH], BF, tag=f"ro{l}")
                    nc.gpsimd.collective_compute(
                        "ReduceScatter", mybir.AluOpType.add,
                        replica_groups=[list(range(NCORES))],
                        ins=[rs_in[:].opt()], outs=[rs_out[:].opt()])

                    # --- hV + bias for own nodes (overlaps the collective)
                    hv_sb = hvp.tile([P, NBL * CH], BF, tag="hv")
                    for b in range(NBL):
                        v_ps = pshv.tile([P, CH], F32, tag="hv")
                        for g in range(kg):
                            nc.tensor.matmul(
                                v_ps[:],
                                lhsT_t[:, g * NPAD + b * P:g * NPAD + (b + 1) * P],
                                vt[:, g * CH:(g + 1) * CH],
                                start=(g == 0), stop=(g == kg - 1))
                        nc.vector.tensor_add(
                            v_ps[:], v_ps[:], ball_t[:, l * CH:(l + 1) * CH])
                        nc.vector.tensor_copy(hv_sb[:, b * CH:(b + 1) * CH],
                                              v_ps[:])

                    # --- epilogue: h' = relu(agg + hV + b); transpose to hT
                    hT_next = htp.tile([P, KGC * NPAD], BF, tag="hT")
                    for b in range(NBL):
                        agg_t = aggp.tile([P, CH], BF, tag="agg_in")
                        nc.sync.dma_start(agg_t[:], rs_out[b * P:(b + 1) * P, :])
                        h_bf = hp.tile([P, CH], BF, tag="h")
                        nc.vector.tensor_add(
                            h_bf[:], agg_t[:], hv_sb[:, b * CH:(b + 1) * CH])
                        nc.scalar.activation(
                            h_bf[:], h_bf[:], mybir.ActivationFunctionType.Relu)
                        for cg in range(KGC):
                            tr_ps = pstr.tile([P, P], BF, tag="tr")
                            nc.tensor.transpose(
                                tr_ps[:], h_bf[:, cg * P:(cg + 1) * P], ident[:])
                            nc.vector.tensor_copy(
                                hT_next[:, cg * NPAD + b * P:cg * NPAD + (b + 1) * P],
                                tr_ps[:])
                    hT_cur = hT_next

                # ---- final dense: logits = h7 @ Wd + bd
                for b in range(NBL):
                    o_sb = op.tile([P, N_LABELS], F32, tag="o")
                    fps = []
                    for c in range(3):
                        fin_ps = psagg.tile([P, FIN_CHUNK], F32, tag="agg")
                        fps.append(fin_ps)
                    for g in range(KGC):
                        for c in range(3):
                            nc.tensor.matmul(
                                fps[c][:],
                                hT_cur[:, g * NPAD + b * P:g * NPAD + (b + 1) * P],
                                wd_t[:, g * N_LABELS + c * FIN_CHUNK:
                                     g * N_LABELS + (c + 1) * FIN_CHUNK],
                                start=(g == 0), stop=(g == KGC - 1))
                    for c in range(3):
                        sl = slice(c * FIN_CHUNK, (c + 1) * FIN_CHUNK)
                        nc.vector.tensor_add(fps[c][:], fps[c][:], bdr_t[:, sl])
                        nc.scalar.activation(
                            o_sb[:, sl], fps[c][:],
                            mybir.ActivationFunctionType.Copy)
                    if rep == repeat - 1:
                        nc.sync.dma_start(out[b * P:(b + 1) * P, :], o_sb[:])

    _split_excess_waits(nc)
    lower_extended_insts(nc)
    return nc


# ------------------------------------------------------------- entry point
def kernel(x, src, dst, W1, V1, b1, Wk, Vk, bk, Wd, bd, _repeat=1, _nc_cache={}):
    x = np.asarray(x, np.float32)
    kgrp, idx_tabs, sel_tabs = _prep_edges(src, dst)

    key = (tuple(kgrp), _repeat)
    if key not in _nc_cache:
        _nc_cache[key] = _build(kgrp, repeat=_repeat)
    nc = _nc_cache[key]

    # weights (replicated, host-packed)
    w1p = _pack_rhs(np.asarray(W1, np.float32), KG1, CH).astype(BFNP)
    v1p = _pack_rhs(np.asarray(V1, np.float32), KG1, CH).astype(BFNP)
    wkp = np.concatenate(
        [_pack_rhs(np.asarray(Wk[i], np.float32), KGC, CH) for i in range(6)],
        axis=1).astype(BFNP)
    vkp = np.concatenate(
        [_pack_rhs(np.asarray(Vk[i], np.float32), KGC, CH) for i in range(6)],
        axis=1).astype(BFNP)
    wdp = _pack_rhs(np.asarray(Wd, np.float32), KGC, N_LABELS).astype(BFNP)
    ballv = np.concatenate(
        [np.asarray(b1, np.float32)] + [np.asarray(bk[i], np.float32)
                                        for i in range(6)])
    ballp = np.broadcast_to(ballv, (P, NLAYERS * CH)).copy()
    bdp = np.broadcast_to(np.asarray(bd, np.float32), (P, N_LABELS)).copy()

    in_maps = []
    for p in range(NCORES):
        xp = np.zeros((NPAD, IN_F), np.float32)
        xp[:NPC] = x[p * NPC:(p + 1) * NPC]
        xTp = _pack_lhsT(np.ascontiguousarray(xp.T), KG1).astype(BFNP)
        in_maps.append({
            "xT": xTp, "idx": idx_tabs[p], "sel": sel_tabs[p],
            "w1": w1p, "v1": v1p, "wk": wkp, "vk": vkp, "wd": wdp,
            "ball": ballp, "bdr": bdp,
        })

    res = run_bass_kernel_spmd(nc, in_maps, core_ids=list(range(NCORES)))
    outp = np.empty((N_NODES, N_LABELS), np.float32)
    for p in range(NCORES):
        outp[p * NPC:(p + 1) * NPC] = res.results[p]["out"][:NPC]
    return outp


# revision 28
# speedup vs baseline: 2.9572x; 1.3223x over previous
"""ARMA-style GNN message passing on 8 TRN2 NeuronCores.

Reference computation (per layer, 7 layers):
    m   = h @ W                                  [N, CH]
    agg = segment_sum(w[:,None] * m[dst], src)   [N, CH]
    h'  = relu(agg + h @ V + b)
then logits = h @ Wd + bd.

Strategy (dst-partitioned edges + ReduceScatter):
  - 8 cores own 1250 nodes each (padded to 1280 = 10 blocks of 128).
  - Edge (s, d) is processed by the core owning d: the message row m[d]
    lives in that core's local m table, so gathers are local (no
    AllGather of m). Messages are scatter-added into per-src-block
    partial aggregates via PE matmuls with host-built bf16 "selection"
    matrices (segment-sum == sel.T @ msg_rows).
  - Host sorts each core's edges by global src block (80 blocks of 128
    padded-global rows); per (core, src block) the edge count is padded
    to a multiple of 128; the per-block count is the max over cores so
    the SPMD program is identical.
  - Per layer: m = h @ W for own nodes (PE) -> staged to local DRAM;
    batched dma_gather calls (CHUNK edge blocks each, ~1KB rows) pull
    per-edge message rows; per src block the sel matmuls accumulate
    into PSUM; partials staged to DRAM; one ReduceScatter sums the
    [80*128, CH] partials and hands each core its own 10 blocks.
    h@V + bias runs on PE during the ReduceScatter; epilogue adds
    agg + hV, applies relu, and PE-transposes into the next layer's
    stationary operand hT.
  - Final dense layer per core; host concatenates.

All matmuls run in bf16 with fp32 PSUM accumulation.
"""
import numpy as np
import ml_dtypes

import concourse.bass as bass
import concourse.tile as tile
from concourse.tile import add_dep_helper
import concourse.mybir as mybir
from concourse.vector_clock import ScopedClock
from concourse.bass_utils import run_bass_kernel_spmd
from concourse.masks import make_identity
from concourse import library_config
from concourse.library_overlay import lower_extended_insts

# ---------------------------------------------------------------- constants
N_NODES = 10000
N_EDGES = 160000
IN_F = 256
CH = 512
N_LABELS = 1440
NCORES = 8
NPC = N_NODES // NCORES      # 1250 nodes per core
P = 128
NBL = 10                     # node blocks per core (10*128 = 1280)
NPAD = NBL * P               # padded nodes per core
NSB = NCORES * NBL           # 80 global src blocks
NLAYERS = 7
KG1 = IN_F // P              # 2 contraction blocks in layer 1
KGC = CH // P                # 4 contraction blocks in layers 2..7
FIN_CHUNK = 480              # 1440 = 3 * 480, fits one PSUM bank in f32
CHUNK = 8                    # edge blocks per dma_gather call

BF = mybir.dt.bfloat16
F8 = mybir.dt.float8e4          # e4m3: message table / gather payload
F32 = mybir.dt.float32
BFNP = ml_dtypes.bfloat16


# ------------------------------------------------------- walrus workarounds
def _patched_drain_and_barrier(self, tick_clock, wait_clock):
    # This walrus build rejects >1-2 sync waits on one TPB_CTRL; put the
    # kernel-tail drain's waits on separate preceding SP nops instead.
    nc = self.nc
    probe = nc.sync.nop(nofuse=True, hint="drain_waits")
    wait_clock.add_sem_waits(probe.ins, ScopedClock({None: tick_clock.global_clock}))
    si = probe.ins.sync_info
    waits = list(si.on_wait) if si is not None else []
    if len(waits) > 1:
        si.on_wait = waits[:1]
        for i in range(1, len(waits)):
            n2 = nc.sync.nop(nofuse=True, hint=f"drain_waits_{i}")
            n2.ins.sync_info = mybir.SyncInfo(on_wait=[waits[i]], on_update=[])
    nc.sync.drain()
    nc.all_engine_barrier()
    assert self.sems is not None
    popped = nc._tile_sem_poison_stack.pop()
    assert popped is self._sem_poison
    nc.clear_and_free_semaphores(list(self.sems.allocated().values()))
    nc.all_engine_barrier()


tile.TileContext._drain_and_barrier = _patched_drain_and_barrier


def _split_excess_waits(nc, limit=1):
    # Same ISA restriction for ordinary instructions: hoist excess sync
    # waits onto injected same-engine nops placed just before.
    for func in nc.m.functions:
        for bb in func.blocks:
            out = []
            for ins in bb.instructions:
                si = ins.sync_info
                if si is not None and si.on_wait and len(si.on_wait) > limit:
                    waits = list(si.on_wait)
                    excess, keep = waits[:-limit], waits[-limit:]
                    for i in range(0, len(excess), limit):
                        out.append(mybir.InstNoOp(
                            name=f"{ins.name}_xw{i}",
                            engine=ins.engine,
                            ins=[], outs=[],
                            sync_info=mybir.SyncInfo(
                                on_wait=excess[i:i + limit], on_update=[]),
                        ))
                    si.on_wait = keep
                out.append(ins)
            bb.instructions[:] = out


# ------------------------------------------------------------- host prep
def _prep_edges(src, dst):
    """Partition edges by owning core of dst; per core group edges by
    global src block, reordered so each core's first 5 node blocks come
    first (split ReduceScatter halves). Returns (kgrp, idx_tabs, sel_tabs):
      kgrp[g]    edge-block count of src group g in split order
      idx_tabs[p] int16 [128, NEB*8] 16-wrapped local m-row gather indices
      sel_tabs[p] bf16 [128, NEB*128] selection/weight matrices
    """
    src = np.asarray(src).astype(np.int64)
    dst = np.asarray(dst).astype(np.int64)
    deg_out = np.maximum(np.bincount(src, minlength=N_NODES), 1.0).astype(np.float32)
    deg_in = np.maximum(np.bincount(dst, minlength=N_NODES), 1.0).astype(np.float32)
    w = (1.0 / np.sqrt(deg_out[src] * deg_in[dst])).astype(np.float32)

    core = dst // NPC                       # owning core (dst side)
    lrow = (dst % NPC).astype(np.int64)     # gather row in local m table
    g_src = (src // NPC) * NPAD + (src % NPC)
    sb = g_src // P                         # global src block 0..79
    scol = g_src % P                        # column within sel matrix

    order = np.lexsort((sb, core))
    core_s, sb_s = core[order], sb[order]
    lrow_s, scol_s, w_s = lrow[order], scol[order], w[order]
    counts = np.zeros((NCORES, NSB), np.int64)
    np.add.at(counts, (core_s, sb_s), 1)
    # split order: group g -> src block: first all (q, b<5), then (q, b>=5)
    GORDER = ([q * NBL + b for q in range(NCORES) for b in range(NBL // 2)]
              + [q * NBL + b for q in range(NCORES)
                 for b in range(NBL // 2, NBL)])
    kgrp = [max(1, int(-(-counts[:, g].max() // P))) for g in GORDER]
    neb = sum(kgrp)

    idx_tabs, sel_tabs = [], []
    starts = np.zeros((NCORES, NSB), np.int64)
    flat = counts.ravel().cumsum()
    starts.ravel()[1:] = flat[:-1]
    for p in range(NCORES):
        flat_idx = np.zeros(neb * P, np.int64)      # slot -> local m row
        sel_t = np.zeros((P, neb * P), np.float32)
        col = 0
        for gi, b in enumerate(GORDER):
            s0, cnt = starts[p, b], counts[p, b]
            g = lrow_s[s0:s0 + cnt]
            c = scol_s[s0:s0 + cnt]
            ww = w_s[s0:s0 + cnt]
            for k in range(kgrp[gi]):
                lo, hi = k * P, min((k + 1) * P, cnt)
                if hi > lo:
                    lanes = np.arange(hi - lo)
                    flat_idx[col * P + lanes] = g[lo:hi]
                    sel_t[lanes, col * P + c[lo:hi]] = ww[lo:hi]
                col += 1
        # 16-wrap per gather call of CHUNK blocks: slot i of call c lands
        # at [i%16, c*CHUNK*8 + i//16], replicated across the 8 Q7 cores'
        # partition groups (each Q7 core reads its own 16 partitions)
        idx_t = np.zeros((P, neb * 8), np.int16)
        gidx = np.arange(neb * P)
        call = gidx // (CHUNK * P)
        i_in = gidx % (CHUNK * P)
        for q7 in range(8):
            idx_t[16 * q7 + i_in % 16, call * CHUNK * 8 + i_in // 16] = flat_idx
        idx_tabs.append(idx_t)
        sel_tabs.append(sel_t.astype(BFNP))
    return kgrp, idx_tabs, sel_tabs


def _pack_lhsT(xT, kg):
    """[kg*128, NPAD] -> [128, kg*NPAD] (partition-major kg blocks)."""
    return np.ascontiguousarray(
        xT.reshape(kg, P, NPAD).transpose(1, 0, 2).reshape(P, kg * NPAD))


def _pack_rhs(Wm, kg, n):
    """[kg*128, n] -> [128, kg*n]."""
    return np.ascontiguousarray(
        Wm.reshape(kg, P, n).transpose(1, 0, 2).reshape(P, kg * n))


# ------------------------------------------------------------- device build
def _build(kgrp, repeat=1):
    neb = sum(kgrp)
    ncalls = -(-neb // CHUNK)
    nc = bass.Bass("TRN2", target_bir_lowering=False, debug=False,
                   num_devices=NCORES)

    def din(name, shape, dt):
        return nc.dram_tensor(name, shape, dt, kind="ExternalInput").ap()

    xT = din("xT", [P, KG1 * NPAD], BF)
    idx = din("idx", [P, neb * 8], mybir.dt.int16)
    sel = din("sel", [P, neb * P], BF)
    w1 = din("w1", [P, KG1 * CH], BF)
    v1 = din("v1", [P, KG1 * CH], BF)
    wk = din("wk", [P, 6 * KGC * CH], BF)
    vk = din("vk", [P, 6 * KGC * CH], BF)
    wd = din("wd", [P, KGC * N_LABELS], BF)
    ball = din("ball", [P, NLAYERS * CH], F32)
    bdr = din("bdr", [P, N_LABELS], F32)
    out = nc.dram_tensor("out", [NPAD, N_LABELS], F32, kind="ExternalOutput").ap()

    with tile.TileContext(nc) as tc:
        with (
            tc.tile_pool(name="const", bufs=1) as cp,
            tc.tile_pool(name="wv", bufs=2) as wvp,
            tc.tile_pool(name="ht", bufs=2) as htp,
            tc.tile_pool(name="mout", bufs=3) as mp,
            tc.tile_pool(name="msg", bufs=4) as msgp,
            tc.tile_pool(name="part", bufs=10) as prp,
            tc.tile_pool(name="hvs", bufs=2) as hvp,
            tc.tile_pool(name="aggin", bufs=2) as aggp,
            tc.tile_pool(name="hact", bufs=2) as hp,
            tc.tile_pool(name="outs", bufs=1) as op,
            tc.tile_pool(name="psm", bufs=2, space="PSUM") as psm,
            tc.tile_pool(name="psagg", bufs=3, space="PSUM") as psagg,
            tc.tile_pool(name="pshv", bufs=1, space="PSUM") as pshv,
            tc.tile_pool(name="pstr", bufs=2, space="PSUM") as pstr,
            tc.tile_pool(name="dram", bufs=1, space="DRAM") as dram,
        ):
            nc.gpsimd.load_library(library_config.mlp)
            # ---- constants to SBUF
            xT_t = cp.tile([P, KG1 * NPAD], BF)
            nc.sync.dma_start(xT_t[:], xT[:])
            idx_t = cp.tile([P, neb * 8], mybir.dt.int16)
            nc.sync.dma_start(idx_t[:], idx[:])
            sel_t = cp.tile([P, neb * P], BF)
            nc.sync.dma_start(sel_t[:], sel[:])
            w1_t = cp.tile([P, KG1 * CH], BF)
            nc.sync.dma_start(w1_t[:], w1[:])
            v1_t = cp.tile([P, KG1 * CH], BF)
            nc.sync.dma_start(v1_t[:], v1[:])
            wd_t = cp.tile([P, KGC * N_LABELS], BF)
            nc.sync.dma_start(wd_t[:], wd[:])
            ball_t = cp.tile([P, NLAYERS * CH], F32)
            nc.sync.dma_start(ball_t[:], ball[:])
            bdr_t = cp.tile([P, N_LABELS], F32)
            nc.sync.dma_start(bdr_t[:], bdr[:])
            ident = cp.tile([P, P], BF)
            make_identity(nc, ident[:])

            # num_idxs registers for the batched gathers (to_reg on an int
            # immediate has no free-register pool under TileContext)
            rem = neb % CHUNK
            nir_full = nc.gpsimd.alloc_register("nir_full")
            nc.gpsimd.reg_mov(nir_full, CHUNK * P)
            nir_rem = None
            if rem:
                nir_rem = nc.gpsimd.alloc_register("nir_rem")
                nc.gpsimd.reg_mov(nir_rem, rem * P)

            for rep in range(repeat):
                hT_cur = None
                for l in range(NLAYERS):
                    kg = KG1 if l == 0 else KGC
                    if l == 0:
                        lhsT_t = xT_t
                        wt = w1_t[:, :]
                        vt = v1_t[:, :]
                    else:
                        lhsT_t = hT_cur
                        wt_t = wvp.tile([P, KGC * CH], BF, tag="wk")
                        nc.sync.dma_start(
                            wt_t[:], wk[:, (l - 1) * KGC * CH:l * KGC * CH])
                        vt_t = wvp.tile([P, KGC * CH], BF, tag="vk")
                        nc.sync.dma_start(
                            vt_t[:], vk[:, (l - 1) * KGC * CH:l * KGC * CH])
                        wt = wt_t[:, :]
                        vt = vt_t[:, :]

                    # --- m = h @ W for own nodes; stage to local DRAM (fp8)
                    m_dram = dram.tile([NPAD, CH], F8, tag=f"m{l}")
                    for b in range(NBL):
                        m_ps = psm.tile([P, CH], F32, tag="m")
                        for g in range(kg):
                            nc.tensor.matmul(
                                m_ps[:],
                                lhsT_t[:, g * NPAD + b * P:g * NPAD + (b + 1) * P],
                                wt[:, g * CH:(g + 1) * CH],
                                start=(g == 0), stop=(g == kg - 1))
                        m_bf = mp.tile([P, CH], F8, tag="mbf")
                        nc.vector.tensor_copy(m_bf[:], m_ps[:])
                        nc.sync.dma_start(m_dram[b * P:(b + 1) * P, :], m_bf[:])

                    # --- gathers (batched) + scatter matmuls, interleaved,
                    #     in two halves with a ReduceScatter per half so the
                    #     first collective overlaps the second half's work
                    NG2 = NSB // 2
                    rs_in_a = dram.tile([NG2 * P, CH], BF, tag=f"ri{l}h0")
                    rs_in_b = dram.tile([NG2 * P, CH], BF, tag=f"ri{l}h1")
                    rs_out_a = dram.tile([NPAD // 2, CH], BF, tag=f"ro{l}h0")
                    rs_out_b = dram.tile([NPAD // 2, CH], BF, tag=f"ro{l}h1")
                    rs_in = [rs_in_a, rs_in_b]
                    rs_out = [rs_out_a, rs_out_b]
                    msg_tiles = [None] * ncalls
                    col = 0
                    for gi in range(NSB):
                        half, gx = gi // NG2, gi % NG2
                        h_ps = psagg.tile([P, CH], F32, tag="agg")
                        for k in range(kgrp[gi]):
                            ci, wi = col // CHUNK, col % CHUNK
                            if wi == 0:
                                nblk = min(CHUNK, neb - ci * CHUNK)
                                mt = msgp.tile([P, CHUNK * CH], F8, tag="msg")
                                out3 = mt[:].rearrange(
                                    "p (k c) -> p k c", c=CH)[:, :nblk, :]
                                nc.gpsimd.dma_gather(
                                    out_ap=out3,
                                    in_ap=m_dram[:, :],
                                    idxs_ap=idx_t[:, ci * CHUNK * 8:
                                                  ci * CHUNK * 8 + nblk * 8],
                                    num_idxs=nblk * P,
                                    num_idxs_reg=(nir_full if nblk == CHUNK
                                                  else nir_rem),
                                    elem_size=CH, single_packet=False)
                                msg_tiles[ci] = mt
                            mt = msg_tiles[ci]
                            nc.tensor.matmul(
                                h_ps[:],
                                sel_t[:, col * P:(col + 1) * P],
                                mt[:, wi * CH:(wi + 1) * CH],
                                start=(k == 0), stop=(k == kgrp[gi] - 1))
                            col += 1
                        pr_bf = prp.tile([P, CH], BF, tag="pr")
                        last_copy = nc.scalar.activation(
                            pr_bf[:], h_ps[:],
                            mybir.ActivationFunctionType.Copy)
                        last_store = nc.sync.dma_start(
                            rs_in[half][gx * P:(gx + 1) * P, :], pr_bf[:])
                        if gi == NG2 - 1:
                            nc.gpsimd.collective_compute(
                                "ReduceScatter", mybir.AluOpType.add,
                                replica_groups=[list(range(NCORES))],
                                ins=[rs_in[0][:].opt()],
                                outs=[rs_out[0][:].opt()])
                    nc.gpsimd.collective_compute(
                        "ReduceScatter", mybir.AluOpType.add,
                        replica_groups=[list(range(NCORES))],
                        ins=[rs_in[1][:].opt()], outs=[rs_out[1][:].opt()])

                    # --- hV + bias for own nodes (overlaps RS_b)
                    hv_sb = hvp.tile([P, NBL * CH], BF, tag="hv")
                    for b in range(NBL):
                        v_ps = pshv.tile([P, CH], F32, tag="hv")
                        for g in range(kg):
                            nc.tensor.matmul(
                                v_ps[:],
                                lhsT_t[:, g * NPAD + b * P:g * NPAD + (b + 1) * P],
                                vt[:, g * CH:(g + 1) * CH],
                                start=(g == 0), stop=(g == kg - 1))
                        nc.vector.tensor_add(
                            v_ps[:], v_ps[:], ball_t[:, l * CH:(l + 1) * CH])
                        nc.vector.tensor_copy(hv_sb[:, b * CH:(b + 1) * CH],
                                              v_ps[:])

                    # --- epilogue: h' = relu(agg + hV + b); transpose to hT
                    #     (blocks 0..4 need only RS_a, so they run during RS_b)
                    hT_next = htp.tile([P, KGC * NPAD], BF, tag="hT")
                    for b in range(NBL):
                        half, bx = b // (NBL // 2), b % (NBL // 2)
                        agg_t = aggp.tile([P, CH], BF, tag="agg_in")
                        i_ld = nc.sync.dma_start(
                            agg_t[:], rs_out[half][bx * P:(bx + 1) * P, :])
                        h_bf = hp.tile([P, CH], BF, tag="h")
                        i_ad = nc.vector.tensor_add(
                            h_bf[:], agg_t[:], hv_sb[:, b * CH:(b + 1) * CH])
                        i_rl = nc.scalar.activation(
                            h_bf[:], h_bf[:], mybir.ActivationFunctionType.Relu)
                        if b == 0:
                            # scheduler ordering hints: keep the epilogue
                            # behind the second half's copies/stores so it
                            # can't head-of-line-block engines on RS_a
                            add_dep_helper(i_ld.ins, last_store.ins,
                                           reason="epilogue after h2 stores")
                            add_dep_helper(i_ad.ins, last_copy.ins,
                                           reason="epilogue after h2 copies")
                            add_dep_helper(i_rl.ins, last_copy.ins,
                                           reason="epilogue after h2 copies")
                        for cg in range(KGC):
                            tr_ps = pstr.tile([P, P], BF, tag="tr")
                            i_tr = nc.tensor.transpose(
                                tr_ps[:], h_bf[:, cg * P:(cg + 1) * P], ident[:])
                            if b == 0 and cg == 0:
                                add_dep_helper(i_tr.ins, last_copy.ins,
                                               reason="epilogue after h2")
                            nc.vector.tensor_copy(
                                hT_next[:, cg * NPAD + b * P:cg * NPAD + (b + 1) * P],
                                tr_ps[:])
                    hT_cur = hT_next

                # ---- final dense: logits = h7 @ Wd + bd
                for b in range(NBL):
                    o_sb = op.tile([P, N_LABELS], F32, tag="o")
                    fps = []
                    for c in range(3):
                        fin_ps = psagg.tile([P, FIN_CHUNK], F32, tag="agg")
                        fps.append(fin_ps)
                    for g in range(KGC):
                        for c in range(3):
                            nc.tensor.matmul(
                                fps[c][:],
                                hT_cur[:, g * NPAD + b * P:g * NPAD + (b + 1) * P],
                                wd_t[:, g * N_LABELS + c * FIN_CHUNK:
                                     g * N_LABELS + (c + 1) * FIN_CHUNK],
                                start=(g == 0), stop=(g == KGC - 1))
                    for c in range(3):
                        sl = slice(c * FIN_CHUNK, (c + 1) * FIN_CHUNK)
                        nc.vector.tensor_add(fps[c][:], fps[c][:], bdr_t[:, sl])
                        nc.scalar.activation(
                            o_sb[:, sl], fps[c][:],
                            mybir.ActivationFunctionType.Copy)
                    if rep == repeat - 1:
                        nc.sync.dma_start(out[b * P:(b + 1) * P, :], o_sb[:])

    _split_excess_waits(nc)
    lower_extended_insts(nc)
    return nc


# ------------------------------------------------------------- entry point
def kernel(x, src, dst, W1, V1, b1, Wk, Vk, bk, Wd, bd, _repeat=1, _nc_cache={}):
    x = np.asarray(x, np.float32)
    kgrp, idx_tabs, sel_tabs = _prep_edges(src, dst)

    key = (tuple(kgrp), _repeat)
    if key not in _nc_cache:
        _nc_cache[key] = _build(kgrp, repeat=_repeat)
    nc = _nc_cache[key]

    # weights (replicated, host-packed)
    w1p = _pack_rhs(np.asarray(W1, np.float32), KG1, CH).astype(BFNP)
    v1p = _pack_rhs(np.asarray(V1, np.float32), KG1, CH).astype(BFNP)
    wkp = np.concatenate(
        [_pack_rhs(np.asarray(Wk[i], np.float32), KGC, CH) for i in range(6)],
        axis=1).astype(BFNP)
    vkp = np.concatenate(
        [_pack_rhs(np.asarray(Vk[i], np.float32), KGC, CH) for i in range(6)],
        axis=1).astype(BFNP)
    wdp = _pack_rhs(np.asarray(Wd, np.float32), KGC, N_LABELS).astype(BFNP)
    ballv = np.concatenate(
        [np.asarray(b1, np.float32)] + [np.asarray(bk[i], np.float32)
                                        for i in range(6)])
    ballp = np.broadcast_to(ballv, (P, NLAYERS * CH)).copy()
    bdp = np.broadcast_to(np.asarray(bd, np.float32), (P, N_LABELS)).copy()

    in_maps = []
    for p in range(NCORES):
        xp = np.zeros((NPAD, IN_F), np.float32)
        xp[:NPC] = x[p * NPC:(p + 1) * NPC]
        xTp = _pack_lhsT(np.ascontiguousarray(xp.T), KG1).astype(BFNP)
        in_maps.append({
            "xT": xTp, "idx": idx_tabs[p], "sel": sel_tabs[p],
            "w1": w1p, "v1": v1p, "wk": wkp, "vk": vkp, "wd": wdp,
            "ball": ballp, "bdr": bdp,
        })

    res = run_bass_kernel_spmd(nc, in_maps, core_ids=list(range(NCORES)))
    outp = np.empty((N_NODES, N_LABELS), np.float32)
    for p in range(NCORES):
        outp[p * NPC:(p + 1) * NPC] = res.results[p]["out"][:NPC]
    return outp


# revision 34
# speedup vs baseline: 3.0849x; 1.0432x over previous
"""ARMA-style GNN message passing on 8 TRN2 NeuronCores.

Reference computation (per layer, 7 layers):
    m   = h @ W                                  [N, CH]
    agg = segment_sum(w[:,None] * m[dst], src)   [N, CH]
    h'  = relu(agg + h @ V + b)
then logits = h @ Wd + bd.

Strategy (dst-partitioned edges + ReduceScatter):
  - 8 cores own 1250 nodes each (padded to 1280 = 10 blocks of 128).
  - Edge (s, d) is processed by the core owning d: the message row m[d]
    lives in that core's local m table, so gathers are local (no
    AllGather of m). Messages are scatter-added into per-src-block
    partial aggregates via PE matmuls with host-built bf16 "selection"
    matrices (segment-sum == sel.T @ msg_rows).
  - Host sorts each core's edges by global src block (80 blocks of 128
    padded-global rows); per (core, src block) the edge count is padded
    to a multiple of 128; the per-block count is the max over cores so
    the SPMD program is identical.
  - Per layer: m = h @ W for own nodes (PE) -> staged to local DRAM;
    batched dma_gather calls (CHUNK edge blocks each, ~1KB rows) pull
    per-edge message rows; per src block the sel matmuls accumulate
    into PSUM; partials staged to DRAM; one ReduceScatter sums the
    [80*128, CH] partials and hands each core its own 10 blocks.
    h@V + bias runs on PE during the ReduceScatter; epilogue adds
    agg + hV, applies relu, and PE-transposes into the next layer's
    stationary operand hT.
  - Final dense layer per core; host concatenates.

All matmuls run in bf16 with fp32 PSUM accumulation.
"""
import numpy as np
import ml_dtypes

import concourse.bass as bass
import concourse.tile as tile
from concourse.tile import add_dep_helper
import concourse.mybir as mybir
from concourse.vector_clock import ScopedClock
from concourse.bass_utils import run_bass_kernel_spmd
from concourse.masks import make_identity
from concourse import library_config
from concourse.library_overlay import lower_extended_insts

# ---------------------------------------------------------------- constants
N_NODES = 10000
N_EDGES = 160000
IN_F = 256
CH = 512
N_LABELS = 1440
NCORES = 8
NPC = N_NODES // NCORES      # 1250 nodes per core
P = 128
NBL = 10                     # node blocks per core (10*128 = 1280)
NPAD = NBL * P               # padded nodes per core
NSB = NCORES * NBL           # 80 global src blocks
NLAYERS = 7
KG1 = IN_F // P              # 2 contraction blocks in layer 1
KGC = CH // P                # 4 contraction blocks in layers 2..7
FIN_CHUNK = 480              # 1440 = 3 * 480, fits one PSUM bank in f32
SPLIT_B = 6                  # node blocks per core in the first RS half
CHUNK = 8                    # edge blocks per dma_gather call

BF = mybir.dt.bfloat16
F8 = mybir.dt.float8e4          # e4m3: message table / gather payload
F32 = mybir.dt.float32
BFNP = ml_dtypes.bfloat16


# ------------------------------------------------------- walrus workarounds
def _patched_drain_and_barrier(self, tick_clock, wait_clock):
    # This walrus build rejects >1-2 sync waits on one TPB_CTRL; put the
    # kernel-tail drain's waits on separate preceding SP nops instead.
    nc = self.nc
    probe = nc.sync.nop(nofuse=True, hint="drain_waits")
    wait_clock.add_sem_waits(probe.ins, ScopedClock({None: tick_clock.global_clock}))
    si = probe.ins.sync_info
    waits = list(si.on_wait) if si is not None else []
    if len(waits) > 1:
        si.on_wait = waits[:1]
        for i in range(1, len(waits)):
            n2 = nc.sync.nop(nofuse=True, hint=f"drain_waits_{i}")
            n2.ins.sync_info = mybir.SyncInfo(on_wait=[waits[i]], on_update=[])
    nc.sync.drain()
    nc.all_engine_barrier()
    assert self.sems is not None
    popped = nc._tile_sem_poison_stack.pop()
    assert popped is self._sem_poison
    nc.clear_and_free_semaphores(list(self.sems.allocated().values()))
    nc.all_engine_barrier()


tile.TileContext._drain_and_barrier = _patched_drain_and_barrier


def _split_excess_waits(nc, limit=1):
    # Same ISA restriction for ordinary instructions: hoist excess sync
    # waits onto injected same-engine nops placed just before.
    for func in nc.m.functions:
        for bb in func.blocks:
            out = []
            for ins in bb.instructions:
                si = ins.sync_info
                if si is not None and si.on_wait and len(si.on_wait) > limit:
                    waits = list(si.on_wait)
                    excess, keep = waits[:-limit], waits[-limit:]
                    for i in range(0, len(excess), limit):
                        out.append(mybir.InstNoOp(
                            name=f"{ins.name}_xw{i}",
                            engine=ins.engine,
                            ins=[], outs=[],
                            sync_info=mybir.SyncInfo(
                                on_wait=excess[i:i + limit], on_update=[]),
                        ))
                    si.on_wait = keep
                out.append(ins)
            bb.instructions[:] = out


# ------------------------------------------------------------- host prep
def _balance_rows(src, dst):
    """Permute rows within each core so every (src block, dst core) edge
    count stays near/below 256 -> fewer padded edge blocks. Returns
    newrow[node] in [0, NPAD) (block-aligned; rows >= count are padding)."""
    core_of = np.arange(N_NODES) // NPC
    dstcore = core_of[dst]
    e = np.zeros((N_NODES, NCORES), np.int32)
    np.add.at(e, (src, dstcore), 1)
    newrow = np.full(N_NODES, -1, np.int64)
    for a in range(NCORES):
        nodes = np.arange(a * NPC, (a + 1) * NPC)
        ev = e[nodes].astype(np.float64)
        order = np.argsort(-ev.sum(1), kind='stable')
        L = np.zeros((NBL, NCORES))
        cntg = np.zeros(NBL, np.int64)
        asg = np.zeros(NPC, np.int64)
        CAP = 253.0
        for idx in order:
            cand = ev[idx]
            best, bestpen = -1, None
            for g in range(NBL):
                if cntg[g] >= P:
                    continue
                newL = L[g] + cand
                pen = (np.maximum(0, newL - CAP).sum(), newL.max(), cntg[g])
                if bestpen is None or pen < bestpen:
                    bestpen, best = pen, g
            L[best] += cand
            asg[idx] = best
            cntg[best] += 1
        rng = np.random.default_rng(a)
        for it in range(4000):
            g1 = int(np.argmax(L.max(1))) if rng.random() < 0.7 \
                else int(rng.integers(NBL))
            q = int(np.argmax(L[g1]))
            if L[g1][q] <= 256:
                break
            members = np.where(asg == g1)[0]
            i1 = members[np.argmax(ev[members, q])]
            room = np.where(cntg < P)[0]
            moved = False
            if len(room):
                g2 = room[np.argmin(L[room, q])]
                if g2 != g1 and (L[g2] + ev[i1]).max() <= 256:
                    L[g1] -= ev[i1]; L[g2] += ev[i1]
                    cntg[g1] -= 1; cntg[g2] += 1; asg[i1] = g2
                    moved = True
            if not moved:
                g2 = int(np.argmin(L[:, q]))
                mem2 = np.where(asg == g2)[0]
                i2 = mem2[np.argmin(ev[mem2, q])]
                d1, d2 = ev[i1], ev[i2]
                nl1, nl2 = L[g1] - d1 + d2, L[g2] - d2 + d1
                if max(nl1.max(), nl2.max()) < max(L[g1].max(), L[g2].max()):
                    L[g1], L[g2] = nl1, nl2
                    asg[i1], asg[i2] = g2, g1
        fill = np.zeros(NBL, np.int64)
        for j in range(NPC):
            g = asg[j]
            newrow[nodes[j]] = g * P + fill[g]
            fill[g] += 1
    return newrow


def _prep_edges(src, dst):
    """Partition edges by owning core of dst; per core group edges by
    global src block, reordered so each core's first 5 node blocks come
    first (split ReduceScatter halves). Returns (kgrp, idx_tabs, sel_tabs):
      kgrp[g]    edge-block count of src group g in split order
      idx_tabs[p] int16 [128, NEB*8] 16-wrapped local m-row gather indices
      sel_tabs[p] bf16 [128, NEB*128] selection/weight matrices
    """
    src = np.asarray(src).astype(np.int64)
    dst = np.asarray(dst).astype(np.int64)
    deg_out = np.maximum(np.bincount(src, minlength=N_NODES), 1.0).astype(np.float32)
    deg_in = np.maximum(np.bincount(dst, minlength=N_NODES), 1.0).astype(np.float32)
    w = (1.0 / np.sqrt(deg_out[src] * deg_in[dst])).astype(np.float32)

    newrow = _balance_rows(src, dst)        # in-core row permutation
    core = dst // NPC                       # owning core (dst side)
    lrow = newrow[dst]                      # gather row in local m table
    g_src = (src // NPC) * NPAD + newrow[src]
    sb = g_src // P                         # global src block 0..79
    scol = g_src % P                        # column within sel matrix

    order = np.lexsort((sb, core))
    core_s, sb_s = core[order], sb[order]
    lrow_s, scol_s, w_s = lrow[order], scol[order], w[order]
    counts = np.zeros((NCORES, NSB), np.int64)
    np.add.at(counts, (core_s, sb_s), 1)
    # split order: group g -> src block: first all (q, b<SPLIT_B), rest after
    GORDER = ([q * NBL + b for q in range(NCORES) for b in range(SPLIT_B)]
              + [q * NBL + b for q in range(NCORES)
                 for b in range(SPLIT_B, NBL)])
    kgrp = [max(1, int(-(-counts[:, g].max() // P))) for g in GORDER]
    neb = sum(kgrp)

    idx_tabs, sel_tabs = [], []
    starts = np.zeros((NCORES, NSB), np.int64)
    flat = counts.ravel().cumsum()
    starts.ravel()[1:] = flat[:-1]
    for p in range(NCORES):
        flat_idx = np.zeros(neb * P, np.int64)      # slot -> local m row
        sel_t = np.zeros((P, neb * P), np.float32)
        col = 0
        for gi, b in enumerate(GORDER):
            s0, cnt = starts[p, b], counts[p, b]
            g = lrow_s[s0:s0 + cnt]
            c = scol_s[s0:s0 + cnt]
            ww = w_s[s0:s0 + cnt]
            for k in range(kgrp[gi]):
                lo, hi = k * P, min((k + 1) * P, cnt)
                if hi > lo:
                    lanes = np.arange(hi - lo)
                    flat_idx[col * P + lanes] = g[lo:hi]
                    sel_t[lanes, col * P + c[lo:hi]] = ww[lo:hi]
                col += 1
        # 16-wrap per gather call of CHUNK blocks: slot i of call c lands
        # at [i%16, c*CHUNK*8 + i//16], replicated across the 8 Q7 cores'
        # partition groups (each Q7 core reads its own 16 partitions)
        idx_t = np.zeros((P, neb * 8), np.int16)
        gidx = np.arange(neb * P)
        call = gidx // (CHUNK * P)
        i_in = gidx % (CHUNK * P)
        for q7 in range(8):
            idx_t[16 * q7 + i_in % 16, call * CHUNK * 8 + i_in // 16] = flat_idx
        idx_tabs.append(idx_t)
        sel_tabs.append(sel_t.astype(BFNP))
    return kgrp, idx_tabs, sel_tabs, newrow


def _pack_lhsT(xT, kg):
    """[kg*128, NPAD] -> [128, kg*NPAD] (partition-major kg blocks)."""
    return np.ascontiguousarray(
        xT.reshape(kg, P, NPAD).transpose(1, 0, 2).reshape(P, kg * NPAD))


def _pack_rhs(Wm, kg, n):
    """[kg*128, n] -> [128, kg*n]."""
    return np.ascontiguousarray(
        Wm.reshape(kg, P, n).transpose(1, 0, 2).reshape(P, kg * n))


# ------------------------------------------------------------- device build
def _build(kgrp, repeat=1):
    neb = sum(kgrp)
    ncalls = -(-neb // CHUNK)
    nc = bass.Bass("TRN2", target_bir_lowering=False, debug=False,
                   num_devices=NCORES)

    def din(name, shape, dt):
        return nc.dram_tensor(name, shape, dt, kind="ExternalInput").ap()

    xT = din("xT", [P, KG1 * NPAD], BF)
    idx = din("idx", [P, neb * 8], mybir.dt.int16)
    sel = din("sel", [P, neb * P], BF)
    w1 = din("w1", [P, KG1 * CH], BF)
    v1 = din("v1", [P, KG1 * CH], BF)
    wk = din("wk", [P, 6 * KGC * CH], BF)
    vk = din("vk", [P, 6 * KGC * CH], BF)
    wd = din("wd", [P, KGC * N_LABELS], BF)
    ball = din("ball", [P, NLAYERS * CH], F32)
    bdr = din("bdr", [P, N_LABELS], F32)
    out = nc.dram_tensor("out", [NPAD, N_LABELS], F32, kind="ExternalOutput").ap()

    with tile.TileContext(nc) as tc:
        with (
            tc.tile_pool(name="const", bufs=1) as cp,
            tc.tile_pool(name="wv", bufs=2) as wvp,
            tc.tile_pool(name="ht", bufs=2) as htp,
            tc.tile_pool(name="mout", bufs=3) as mp,
            tc.tile_pool(name="msg", bufs=4) as msgp,
            tc.tile_pool(name="part", bufs=10) as prp,
            tc.tile_pool(name="hvs", bufs=2) as hvp,
            tc.tile_pool(name="aggin", bufs=2) as aggp,
            tc.tile_pool(name="hact", bufs=2) as hp,
            tc.tile_pool(name="outs", bufs=1) as op,
            tc.tile_pool(name="psm", bufs=2, space="PSUM") as psm,
            tc.tile_pool(name="psagg", bufs=3, space="PSUM") as psagg,
            tc.tile_pool(name="pshv", bufs=1, space="PSUM") as pshv,
            tc.tile_pool(name="pstr", bufs=2, space="PSUM") as pstr,
            tc.tile_pool(name="dram", bufs=1, space="DRAM") as dram,
        ):
            nc.gpsimd.load_library(library_config.mlp)
            # ---- constants to SBUF
            xT_t = cp.tile([P, KG1 * NPAD], BF)
            nc.sync.dma_start(xT_t[:], xT[:])
            idx_t = cp.tile([P, neb * 8], mybir.dt.int16)
            nc.sync.dma_start(idx_t[:], idx[:])
            sel_t = cp.tile([P, neb * P], BF)
            nc.sync.dma_start(sel_t[:], sel[:])
            w1_t = cp.tile([P, KG1 * CH], BF)
            nc.sync.dma_start(w1_t[:], w1[:])
            v1_t = cp.tile([P, KG1 * CH], BF)
            nc.sync.dma_start(v1_t[:], v1[:])
            wd_t = cp.tile([P, KGC * N_LABELS], BF)
            nc.sync.dma_start(wd_t[:], wd[:])
            ball_t = cp.tile([P, NLAYERS * CH], F32)
            nc.sync.dma_start(ball_t[:], ball[:])
            bdr_t = cp.tile([P, N_LABELS], F32)
            nc.sync.dma_start(bdr_t[:], bdr[:])
            ident = cp.tile([P, P], BF)
            make_identity(nc, ident[:])

            # num_idxs registers for the batched gathers (to_reg on an int
            # immediate has no free-register pool under TileContext)
            rem = neb % CHUNK
            nir_full = nc.gpsimd.alloc_register("nir_full")
            nc.gpsimd.reg_mov(nir_full, CHUNK * P)
            nir_rem = None
            if rem:
                nir_rem = nc.gpsimd.alloc_register("nir_rem")
                nc.gpsimd.reg_mov(nir_rem, rem * P)

            for rep in range(repeat):
                hT_cur = None
                for l in range(NLAYERS):
                    kg = KG1 if l == 0 else KGC
                    if l == 0:
                        lhsT_t = xT_t
                        wt = w1_t[:, :]
                        vt = v1_t[:, :]
                    else:
                        lhsT_t = hT_cur
                        wt_t = wvp.tile([P, KGC * CH], BF, tag="wk")
                        nc.sync.dma_start(
                            wt_t[:], wk[:, (l - 1) * KGC * CH:l * KGC * CH])
                        vt_t = wvp.tile([P, KGC * CH], BF, tag="vk")
                        nc.sync.dma_start(
                            vt_t[:], vk[:, (l - 1) * KGC * CH:l * KGC * CH])
                        wt = wt_t[:, :]
                        vt = vt_t[:, :]

                    # --- m = h @ W for own nodes; stage to local DRAM (fp8)
                    m_dram = dram.tile([NPAD, CH], F8, tag=f"m{l}")
                    for b in range(NBL):
                        m_ps = psm.tile([P, CH], F32, tag="m")
                        for g in range(kg):
                            nc.tensor.matmul(
                                m_ps[:],
                                lhsT_t[:, g * NPAD + b * P:g * NPAD + (b + 1) * P],
                                wt[:, g * CH:(g + 1) * CH],
                                start=(g == 0), stop=(g == kg - 1))
                        m_bf = mp.tile([P, CH], F8, tag="mbf")
                        nc.vector.tensor_copy(m_bf[:], m_ps[:])
                        nc.sync.dma_start(m_dram[b * P:(b + 1) * P, :], m_bf[:])

                    # --- gathers (batched) + scatter matmuls, interleaved,
                    #     in two halves with a ReduceScatter per half so the
                    #     first collective overlaps the second half's work
                    NGA = NCORES * SPLIT_B
                    NGB = NSB - NGA
                    rs_in_a = dram.tile([NGA * P, CH], BF, tag=f"ri{l}h0")
                    rs_in_b = dram.tile([NGB * P, CH], BF, tag=f"ri{l}h1")
                    rs_out_a = dram.tile([SPLIT_B * P, CH], BF, tag=f"ro{l}h0")
                    rs_out_b = dram.tile([(NBL - SPLIT_B) * P, CH], BF,
                                         tag=f"ro{l}h1")
                    rs_in = [rs_in_a, rs_in_b]
                    rs_out = [rs_out_a, rs_out_b]
                    msg_tiles = [None] * ncalls
                    col = 0
                    for gi in range(NSB):
                        half = int(gi >= NGA)
                        gx = gi - NGA if half else gi
                        h_ps = psagg.tile([P, CH], F32, tag="agg")
                        for k in range(kgrp[gi]):
                            ci, wi = col // CHUNK, col % CHUNK
                            if wi == 0:
                                nblk = min(CHUNK, neb - ci * CHUNK)
                                mt = msgp.tile([P, CHUNK * CH], F8, tag="msg")
                                out3 = mt[:].rearrange(
                                    "p (k c) -> p k c", c=CH)[:, :nblk, :]
                                nc.gpsimd.dma_gather(
                                    out_ap=out3,
                                    in_ap=m_dram[:, :],
                                    idxs_ap=idx_t[:, ci * CHUNK * 8:
                                                  ci * CHUNK * 8 + nblk * 8],
                                    num_idxs=nblk * P,
                                    num_idxs_reg=(nir_full if nblk == CHUNK
                                                  else nir_rem),
                                    elem_size=CH, single_packet=False)
                                msg_tiles[ci] = mt
                            mt = msg_tiles[ci]
                            nc.tensor.matmul(
                                h_ps[:],
                                sel_t[:, col * P:(col + 1) * P],
                                mt[:, wi * CH:(wi + 1) * CH],
                                start=(k == 0), stop=(k == kgrp[gi] - 1))
                            col += 1
                        pr_bf = prp.tile([P, CH], BF, tag="pr")
                        last_copy = nc.scalar.activation(
                            pr_bf[:], h_ps[:],
                            mybir.ActivationFunctionType.Copy)
                        last_store = nc.sync.dma_start(
                            rs_in[half][gx * P:(gx + 1) * P, :], pr_bf[:])
                        if gi == NGA - 1:
                            nc.gpsimd.collective_compute(
                                "ReduceScatter", mybir.AluOpType.add,
                                replica_groups=[list(range(NCORES))],
                                ins=[rs_in[0][:].opt()],
                                outs=[rs_out[0][:].opt()])
                    nc.gpsimd.collective_compute(
                        "ReduceScatter", mybir.AluOpType.add,
                        replica_groups=[list(range(NCORES))],
                        ins=[rs_in[1][:].opt()], outs=[rs_out[1][:].opt()])

                    # --- hV + bias for own nodes (overlaps RS_b)
                    hv_sb = hvp.tile([P, NBL * CH], BF, tag="hv")
                    for b in range(NBL):
                        v_ps = pshv.tile([P, CH], F32, tag="hv")
                        for g in range(kg):
                            nc.tensor.matmul(
                                v_ps[:],
                                lhsT_t[:, g * NPAD + b * P:g * NPAD + (b + 1) * P],
                                vt[:, g * CH:(g + 1) * CH],
                                start=(g == 0), stop=(g == kg - 1))
                        nc.vector.tensor_add(
                            v_ps[:], v_ps[:], ball_t[:, l * CH:(l + 1) * CH])
                        nc.vector.tensor_copy(hv_sb[:, b * CH:(b + 1) * CH],
                                              v_ps[:])

                    # --- epilogue: h' = relu(agg + hV + b); transpose to hT
                    #     (blocks 0..4 need only RS_a, so they run during RS_b)
                    hT_next = htp.tile([P, KGC * NPAD], BF, tag="hT")
                    for b in range(NBL):
                        half = int(b >= SPLIT_B)
                        bx = b - SPLIT_B if half else b
                        agg_t = aggp.tile([P, CH], BF, tag="agg_in")
                        i_ld = nc.sync.dma_start(
                            agg_t[:], rs_out[half][bx * P:(bx + 1) * P, :])
                        h_bf = hp.tile([P, CH], BF, tag="h")
                        i_ad = nc.vector.tensor_add(
                            h_bf[:], agg_t[:], hv_sb[:, b * CH:(b + 1) * CH])
                        i_rl = nc.scalar.activation(
                            h_bf[:], h_bf[:], mybir.ActivationFunctionType.Relu)
                        if b == 0:
                            # scheduler ordering hints: keep the epilogue
                            # behind the second half's copies/stores so it
                            # can't head-of-line-block engines on RS_a
                            add_dep_helper(i_ld.ins, last_store.ins,
                                           reason="epilogue after h2 stores")
                            add_dep_helper(i_ad.ins, last_copy.ins,
                                           reason="epilogue after h2 copies")
                            add_dep_helper(i_rl.ins, last_copy.ins,
                                           reason="epilogue after h2 copies")
                        for cg in range(KGC):
                            tr_ps = pstr.tile([P, P], BF, tag="tr")
                            i_tr = nc.tensor.transpose(
                                tr_ps[:], h_bf[:, cg * P:(cg + 1) * P], ident[:])
                            if b == 0 and cg == 0:
                                add_dep_helper(i_tr.ins, last_copy.ins,
                                               reason="epilogue after h2")
                            nc.vector.tensor_copy(
                                hT_next[:, cg * NPAD + b * P:cg * NPAD + (b + 1) * P],
                                tr_ps[:])
                    hT_cur = hT_next

                # ---- final dense: logits = h7 @ Wd + bd
                for b in range(NBL):
                    o_sb = op.tile([P, N_LABELS], F32, tag="o")
                    fps = []
                    for c in range(3):
                        fin_ps = psagg.tile([P, FIN_CHUNK], F32, tag="agg")
                        fps.append(fin_ps)
                    for g in range(KGC):
                        for c in range(3):
                            nc.tensor.matmul(
                                fps[c][:],
                                hT_cur[:, g * NPAD + b * P:g * NPAD + (b + 1) * P],
                                wd_t[:, g * N_LABELS + c * FIN_CHUNK:
                                     g * N_LABELS + (c + 1) * FIN_CHUNK],
                                start=(g == 0), stop=(g == KGC - 1))
                    for c in range(3):
                        sl = slice(c * FIN_CHUNK, (c + 1) * FIN_CHUNK)
                        nc.vector.tensor_add(fps[c][:], fps[c][:], bdr_t[:, sl])
                        nc.scalar.activation(
                            o_sb[:, sl], fps[c][:],
                            mybir.ActivationFunctionType.Copy)
                    if rep == repeat - 1:
                        nc.sync.dma_start(out[b * P:(b + 1) * P, :], o_sb[:])

    _split_excess_waits(nc)
    lower_extended_insts(nc)
    return nc


# ------------------------------------------------------------- entry point
def kernel(x, src, dst, W1, V1, b1, Wk, Vk, bk, Wd, bd, _repeat=1, _nc_cache={}):
    x = np.asarray(x, np.float32)
    kgrp, idx_tabs, sel_tabs, newrow = _prep_edges(src, dst)

    key = (tuple(kgrp), _repeat)
    if key not in _nc_cache:
        _nc_cache[key] = _build(kgrp, repeat=_repeat)
    nc = _nc_cache[key]

    # weights (replicated, host-packed)
    w1p = _pack_rhs(np.asarray(W1, np.float32), KG1, CH).astype(BFNP)
    v1p = _pack_rhs(np.asarray(V1, np.float32), KG1, CH).astype(BFNP)
    wkp = np.concatenate(
        [_pack_rhs(np.asarray(Wk[i], np.float32), KGC, CH) for i in range(6)],
        axis=1).astype(BFNP)
    vkp = np.concatenate(
        [_pack_rhs(np.asarray(Vk[i], np.float32), KGC, CH) for i in range(6)],
        axis=1).astype(BFNP)
    wdp = _pack_rhs(np.asarray(Wd, np.float32), KGC, N_LABELS).astype(BFNP)
    ballv = np.concatenate(
        [np.asarray(b1, np.float32)] + [np.asarray(bk[i], np.float32)
                                        for i in range(6)])
    ballp = np.broadcast_to(ballv, (P, NLAYERS * CH)).copy()
    bdp = np.broadcast_to(np.asarray(bd, np.float32), (P, N_LABELS)).copy()

    in_maps = []
    for p in range(NCORES):
        xp = np.zeros((NPAD, IN_F), np.float32)
        nodes = np.arange(p * NPC, (p + 1) * NPC)
        xp[newrow[nodes]] = x[nodes]
        xTp = _pack_lhsT(np.ascontiguousarray(xp.T), KG1).astype(BFNP)
        in_maps.append({
            "xT": xTp, "idx": idx_tabs[p], "sel": sel_tabs[p],
            "w1": w1p, "v1": v1p, "wk": wkp, "vk": vkp, "wd": wdp,
            "ball": ballp, "bdr": bdp,
        })

    res = run_bass_kernel_spmd(nc, in_maps, core_ids=list(range(NCORES)))
    outp = np.empty((N_NODES, N_LABELS), np.float32)
    for p in range(NCORES):
        nodes = np.arange(p * NPC, (p + 1) * NPC)
        outp[nodes] = res.results[p]["out"][newrow[nodes]]
    return outp


# revision 38
# speedup vs baseline: 3.1044x; 1.0063x over previous
"""ARMA-style GNN message passing on 8 TRN2 NeuronCores.

Reference computation (per layer, 7 layers):
    m   = h @ W                                  [N, CH]
    agg = segment_sum(w[:,None] * m[dst], src)   [N, CH]
    h'  = relu(agg + h @ V + b)
then logits = h @ Wd + bd.

Strategy (dst-partitioned edges + ReduceScatter):
  - 8 cores own 1250 nodes each (padded to 1280 = 10 blocks of 128).
  - Edge (s, d) is processed by the core owning d: the message row m[d]
    lives in that core's local m table, so gathers are local (no
    AllGather of m). Messages are scatter-added into per-src-block
    partial aggregates via PE matmuls with host-built bf16 "selection"
    matrices (segment-sum == sel.T @ msg_rows).
  - Host sorts each core's edges by global src block (80 blocks of 128
    padded-global rows); per (core, src block) the edge count is padded
    to a multiple of 128; the per-block count is the max over cores so
    the SPMD program is identical.
  - Per layer: m = h @ W for own nodes (PE) -> staged to local DRAM;
    batched dma_gather calls (CHUNK edge blocks each, ~1KB rows) pull
    per-edge message rows; per src block the sel matmuls accumulate
    into PSUM; partials staged to DRAM; one ReduceScatter sums the
    [80*128, CH] partials and hands each core its own 10 blocks.
    h@V + bias runs on PE during the ReduceScatter; epilogue adds
    agg + hV, applies relu, and PE-transposes into the next layer's
    stationary operand hT.
  - Final dense layer per core; host concatenates.

All matmuls run in bf16 with fp32 PSUM accumulation.
"""
import numpy as np
import ml_dtypes

import concourse.bass as bass
import concourse.tile as tile
from concourse.tile import add_dep_helper
import concourse.mybir as mybir
from concourse.vector_clock import ScopedClock
from concourse.bass_utils import run_bass_kernel_spmd
from concourse.masks import make_identity
from concourse import library_config
from concourse.library_overlay import lower_extended_insts

# ---------------------------------------------------------------- constants
N_NODES = 10000
N_EDGES = 160000
IN_F = 256
CH = 512
N_LABELS = 1440
NCORES = 8
NPC = N_NODES // NCORES      # 1250 nodes per core
P = 128
NBL = 10                     # node blocks per core (10*128 = 1280)
NPAD = NBL * P               # padded nodes per core
NSB = NCORES * NBL           # 80 global src blocks
NLAYERS = 7
KG1 = IN_F // P              # 2 contraction blocks in layer 1
KGC = CH // P                # 4 contraction blocks in layers 2..7
FIN_CHUNK = 480              # 1440 = 3 * 480, fits one PSUM bank in f32
SPLIT_B = 6                  # node blocks per core in the first RS half
CHUNK = 8                    # edge blocks per dma_gather call

BF = mybir.dt.bfloat16
F8 = mybir.dt.float8e4          # e4m3: message table / gather payload
F32 = mybir.dt.float32
BFNP = ml_dtypes.bfloat16


# ------------------------------------------------------- walrus workarounds
def _patched_drain_and_barrier(self, tick_clock, wait_clock):
    # This walrus build rejects >1-2 sync waits on one TPB_CTRL; put the
    # kernel-tail drain's waits on separate preceding SP nops instead.
    nc = self.nc
    probe = nc.sync.nop(nofuse=True, hint="drain_waits")
    wait_clock.add_sem_waits(probe.ins, ScopedClock({None: tick_clock.global_clock}))
    si = probe.ins.sync_info
    waits = list(si.on_wait) if si is not None else []
    if len(waits) > 1:
        si.on_wait = waits[:1]
        for i in range(1, len(waits)):
            n2 = nc.sync.nop(nofuse=True, hint=f"drain_waits_{i}")
            n2.ins.sync_info = mybir.SyncInfo(on_wait=[waits[i]], on_update=[])
    nc.sync.drain()
    nc.all_engine_barrier()
    assert self.sems is not None
    popped = nc._tile_sem_poison_stack.pop()
    assert popped is self._sem_poison
    nc.clear_and_free_semaphores(list(self.sems.allocated().values()))
    nc.all_engine_barrier()


tile.TileContext._drain_and_barrier = _patched_drain_and_barrier


def _split_excess_waits(nc, limit=1):
    # Same ISA restriction for ordinary instructions: hoist excess sync
    # waits onto injected same-engine nops placed just before.
    for func in nc.m.functions:
        for bb in func.blocks:
            out = []
            for ins in bb.instructions:
                si = ins.sync_info
                if si is not None and si.on_wait and len(si.on_wait) > limit:
                    waits = list(si.on_wait)
                    excess, keep = waits[:-limit], waits[-limit:]
                    for i in range(0, len(excess), limit):
                        out.append(mybir.InstNoOp(
                            name=f"{ins.name}_xw{i}",
                            engine=ins.engine,
                            ins=[], outs=[],
                            sync_info=mybir.SyncInfo(
                                on_wait=excess[i:i + limit], on_update=[]),
                        ))
                    si.on_wait = keep
                out.append(ins)
            bb.instructions[:] = out


# ------------------------------------------------------------- host prep
def _balance_rows(src, dst):
    """Permute rows within each core so every (src block, dst core) edge
    count stays near/below 256 -> fewer padded edge blocks. Returns
    newrow[node] in [0, NPAD) (block-aligned; rows >= count are padding)."""
    core_of = np.arange(N_NODES) // NPC
    dstcore = core_of[dst]
    e = np.zeros((N_NODES, NCORES), np.int32)
    np.add.at(e, (src, dstcore), 1)
    newrow = np.full(N_NODES, -1, np.int64)
    for a in range(NCORES):
        nodes = np.arange(a * NPC, (a + 1) * NPC)
        ev = e[nodes].astype(np.float64)
        order = np.argsort(-ev.sum(1), kind='stable')
        L = np.zeros((NBL, NCORES))
        cntg = np.zeros(NBL, np.int64)
        asg = np.zeros(NPC, np.int64)
        CAP = 253.0
        for idx in order:
            cand = ev[idx]
            best, bestpen = -1, None
            for g in range(NBL):
                if cntg[g] >= P:
                    continue
                newL = L[g] + cand
                pen = (np.maximum(0, newL - CAP).sum(), newL.max(), cntg[g])
                if bestpen is None or pen < bestpen:
                    bestpen, best = pen, g
            L[best] += cand
            asg[idx] = best
            cntg[best] += 1
        rng = np.random.default_rng(a)
        for it in range(4000):
            g1 = int(np.argmax(L.max(1))) if rng.random() < 0.7 \
                else int(rng.integers(NBL))
            q = int(np.argmax(L[g1]))
            if L[g1][q] <= 256:
                break
            members = np.where(asg == g1)[0]
            i1 = members[np.argmax(ev[members, q])]
            room = np.where(cntg < P)[0]
            moved = False
            if len(room):
                g2 = room[np.argmin(L[room, q])]
                if g2 != g1 and (L[g2] + ev[i1]).max() <= 256:
                    L[g1] -= ev[i1]; L[g2] += ev[i1]
                    cntg[g1] -= 1; cntg[g2] += 1; asg[i1] = g2
                    moved = True
            if not moved:
                g2 = int(np.argmin(L[:, q]))
                mem2 = np.where(asg == g2)[0]
                i2 = mem2[np.argmin(ev[mem2, q])]
                d1, d2 = ev[i1], ev[i2]
                nl1, nl2 = L[g1] - d1 + d2, L[g2] - d2 + d1
                if max(nl1.max(), nl2.max()) < max(L[g1].max(), L[g2].max()):
                    L[g1], L[g2] = nl1, nl2
                    asg[i1], asg[i2] = g2, g1
        fill = np.zeros(NBL, np.int64)
        for j in range(NPC):
            g = asg[j]
            newrow[nodes[j]] = g * P + fill[g]
            fill[g] += 1
    return newrow


def _prep_edges(src, dst):
    """Partition edges by owning core of dst; per core group edges by
    global src block, reordered so each core's first 5 node blocks come
    first (split ReduceScatter halves). Returns (kgrp, idx_tabs, sel_tabs):
      kgrp[g]    edge-block count of src group g in split order
      idx_tabs[p] int16 [128, NEB*8] 16-wrapped local m-row gather indices
      sel_tabs[p] bf16 [128, NEB*128] selection/weight matrices
    """
    src = np.asarray(src).astype(np.int64)
    dst = np.asarray(dst).astype(np.int64)
    deg_out = np.maximum(np.bincount(src, minlength=N_NODES), 1.0).astype(np.float32)
    deg_in = np.maximum(np.bincount(dst, minlength=N_NODES), 1.0).astype(np.float32)
    w = (1.0 / np.sqrt(deg_out[src] * deg_in[dst])).astype(np.float32)

    newrow = _balance_rows(src, dst)        # in-core row permutation
    core = dst // NPC                       # owning core (dst side)
    lrow = newrow[dst]                      # gather row in local m table
    g_src = (src // NPC) * NPAD + newrow[src]
    sb = g_src // P                         # global src block 0..79
    scol = g_src % P                        # column within sel matrix

    order = np.lexsort((sb, core))
    core_s, sb_s = core[order], sb[order]
    lrow_s, scol_s, w_s = lrow[order], scol[order], w[order]
    counts = np.zeros((NCORES, NSB), np.int64)
    np.add.at(counts, (core_s, sb_s), 1)
    # split order: group g -> src block: first all (q, b<SPLIT_B), rest after
    GORDER = ([q * NBL + b for q in range(NCORES) for b in range(SPLIT_B)]
              + [q * NBL + b for q in range(NCORES)
                 for b in range(SPLIT_B, NBL)])
    kgrp = [max(1, int(-(-counts[:, g].max() // P))) for g in GORDER]
    neb = sum(kgrp)

    idx_tabs, sel_tabs = [], []
    starts = np.zeros((NCORES, NSB), np.int64)
    flat = counts.ravel().cumsum()
    starts.ravel()[1:] = flat[:-1]
    for p in range(NCORES):
        flat_idx = np.zeros(neb * P, np.int64)      # slot -> local m row
        sel_t = np.zeros((P, neb * P), np.float32)
        col = 0
        for gi, b in enumerate(GORDER):
            s0, cnt = starts[p, b], counts[p, b]
            g = lrow_s[s0:s0 + cnt]
            c = scol_s[s0:s0 + cnt]
            ww = w_s[s0:s0 + cnt]
            for k in range(kgrp[gi]):
                lo, hi = k * P, min((k + 1) * P, cnt)
                if hi > lo:
                    lanes = np.arange(hi - lo)
                    flat_idx[col * P + lanes] = g[lo:hi]
                    sel_t[lanes, col * P + c[lo:hi]] = ww[lo:hi]
                col += 1
        # 16-wrap per gather call of CHUNK blocks: slot i of call c lands
        # at [i%16, c*CHUNK*8 + i//16], replicated across the 8 Q7 cores'
        # partition groups (each Q7 core reads its own 16 partitions)
        idx_t = np.zeros((P, neb * 8), np.int16)
        gidx = np.arange(neb * P)
        call = gidx // (CHUNK * P)
        i_in = gidx % (CHUNK * P)
        for q7 in range(8):
            idx_t[16 * q7 + i_in % 16, call * CHUNK * 8 + i_in // 16] = flat_idx
        idx_tabs.append(idx_t)
        sel_tabs.append(sel_t.astype(BFNP))
    return kgrp, idx_tabs, sel_tabs, newrow


def _pack_lhsT(xT, kg):
    """[kg*128, NPAD] -> [128, kg*NPAD] (partition-major kg blocks)."""
    return np.ascontiguousarray(
        xT.reshape(kg, P, NPAD).transpose(1, 0, 2).reshape(P, kg * NPAD))


def _pack_rhs(Wm, kg, n):
    """[kg*128, n] -> [128, kg*n]."""
    return np.ascontiguousarray(
        Wm.reshape(kg, P, n).transpose(1, 0, 2).reshape(P, kg * n))


# ------------------------------------------------------------- device build
def _build(kgrp, repeat=1):
    neb = sum(kgrp)
    ncalls = -(-neb // CHUNK)
    nc = bass.Bass("TRN2", target_bir_lowering=False, debug=False,
                   num_devices=NCORES)

    def din(name, shape, dt):
        return nc.dram_tensor(name, shape, dt, kind="ExternalInput").ap()

    xT = din("xT", [P, KG1 * NPAD], BF)
    idx = din("idx", [P, neb * 8], mybir.dt.int16)
    sel = din("sel", [P, neb * P], BF)
    w1 = din("w1", [P, KG1 * CH], BF)
    v1 = din("v1", [P, KG1 * CH], BF)
    wk = din("wk", [P, 6 * KGC * CH], BF)
    vk = din("vk", [P, 6 * KGC * CH], BF)
    wd = din("wd", [P, KGC * N_LABELS], BF)
    ball = din("ball", [P, NLAYERS * CH], F32)
    bdr = din("bdr", [P, N_LABELS], F32)
    out = nc.dram_tensor("out", [NPAD, N_LABELS], F32, kind="ExternalOutput").ap()

    with tile.TileContext(nc) as tc:
        with (
            tc.tile_pool(name="const", bufs=1) as cp,
            tc.tile_pool(name="wv", bufs=2) as wvp,
            tc.tile_pool(name="ht", bufs=2) as htp,
            tc.tile_pool(name="mout", bufs=3) as mp,
            tc.tile_pool(name="msg", bufs=4) as msgp,
            tc.tile_pool(name="part", bufs=10) as prp,
            tc.tile_pool(name="hvs", bufs=2) as hvp,
            tc.tile_pool(name="aggin", bufs=2) as aggp,
            tc.tile_pool(name="hact", bufs=2) as hp,
            tc.tile_pool(name="outs", bufs=2) as op,
            tc.tile_pool(name="psm", bufs=2, space="PSUM") as psm,
            tc.tile_pool(name="psagg", bufs=3, space="PSUM") as psagg,
            tc.tile_pool(name="pshv", bufs=1, space="PSUM") as pshv,
            tc.tile_pool(name="pstr", bufs=2, space="PSUM") as pstr,
            tc.tile_pool(name="dram", bufs=1, space="DRAM") as dram,
        ):
            nc.gpsimd.load_library(library_config.mlp)
            # ---- constants to SBUF
            xT_t = cp.tile([P, KG1 * NPAD], BF)
            nc.sync.dma_start(xT_t[:], xT[:])
            idx_t = cp.tile([P, neb * 8], mybir.dt.int16)
            nc.sync.dma_start(idx_t[:], idx[:])
            sel_t = cp.tile([P, neb * P], BF)
            nc.sync.dma_start(sel_t[:], sel[:])
            w1_t = cp.tile([P, KG1 * CH], BF)
            nc.sync.dma_start(w1_t[:], w1[:])
            v1_t = cp.tile([P, KG1 * CH], BF)
            nc.sync.dma_start(v1_t[:], v1[:])
            wd_t = cp.tile([P, KGC * N_LABELS], BF)
            nc.sync.dma_start(wd_t[:], wd[:])
            ball_t = cp.tile([P, NLAYERS * CH], F32)
            nc.sync.dma_start(ball_t[:], ball[:])
            bdr_t = cp.tile([P, N_LABELS], F32)
            nc.sync.dma_start(bdr_t[:], bdr[:])
            ident = cp.tile([P, P], BF)
            make_identity(nc, ident[:])

            # num_idxs registers for the batched gathers (to_reg on an int
            # immediate has no free-register pool under TileContext)
            rem = neb % CHUNK
            nir_full = nc.gpsimd.alloc_register("nir_full")
            nc.gpsimd.reg_mov(nir_full, CHUNK * P)
            nir_rem = None
            if rem:
                nir_rem = nc.gpsimd.alloc_register("nir_rem")
                nc.gpsimd.reg_mov(nir_rem, rem * P)

            for rep in range(repeat):
                # --- prologue: m(0) = x @ W1, staged to local DRAM (fp8)
                m_dram = dram.tile([NPAD, CH], F8, tag="m0")
                for b in range(NBL):
                    m_ps = psm.tile([P, CH], F32, tag="m")
                    for g in range(KG1):
                        nc.tensor.matmul(
                            m_ps[:],
                            xT_t[:, g * NPAD + b * P:g * NPAD + (b + 1) * P],
                            w1_t[:, g * CH:(g + 1) * CH],
                            start=(g == 0), stop=(g == KG1 - 1))
                    m_bf = mp.tile([P, CH], F8, tag="mbf")
                    nc.vector.tensor_copy(m_bf[:], m_ps[:])
                    nc.sync.dma_start(m_dram[b * P:(b + 1) * P, :], m_bf[:])

                hT_cur = None
                vt = v1_t[:, :]
                for l in range(NLAYERS):
                    kg = KG1 if l == 0 else KGC
                    lhsT_t = xT_t if l == 0 else hT_cur
                    if l < NLAYERS - 1:
                        # weights for the NEXT layer: wtn feeds m(l+1) at the
                        # end of this layer; vtn feeds hv(l+1) next layer
                        wt_t = wvp.tile([P, KGC * CH], BF, tag="wk")
                        nc.sync.dma_start(
                            wt_t[:], wk[:, l * KGC * CH:(l + 1) * KGC * CH])
                        vt_t = wvp.tile([P, KGC * CH], BF, tag="vk")
                        nc.sync.dma_start(
                            vt_t[:], vk[:, l * KGC * CH:(l + 1) * KGC * CH])
                        wtn = wt_t[:, :]

                    # --- gathers (batched) + scatter matmuls, interleaved,
                    #     in two halves with a ReduceScatter per half so the
                    #     first collective overlaps the second half's work
                    NGA = NCORES * SPLIT_B
                    NGB = NSB - NGA
                    rs_in_a = dram.tile([NGA * P, CH], BF, tag=f"ri{l}h0")
                    rs_in_b = dram.tile([NGB * P, CH], BF, tag=f"ri{l}h1")
                    rs_out_a = dram.tile([SPLIT_B * P, CH], BF, tag=f"ro{l}h0")
                    rs_out_b = dram.tile([(NBL - SPLIT_B) * P, CH], BF,
                                         tag=f"ro{l}h1")
                    rs_in = [rs_in_a, rs_in_b]
                    rs_out = [rs_out_a, rs_out_b]
                    msg_tiles = [None] * ncalls
                    col = 0
                    for gi in range(NSB):
                        half = int(gi >= NGA)
                        gx = gi - NGA if half else gi
                        h_ps = psagg.tile([P, CH], F32, tag="agg")
                        for k in range(kgrp[gi]):
                            ci, wi = col // CHUNK, col % CHUNK
                            if wi == 0:
                                nblk = min(CHUNK, neb - ci * CHUNK)
                                mt = msgp.tile([P, CHUNK * CH], F8, tag="msg")
                                out3 = mt[:].rearrange(
                                    "p (k c) -> p k c", c=CH)[:, :nblk, :]
                                nc.gpsimd.dma_gather(
                                    out_ap=out3,
                                    in_ap=m_dram[:, :],
                                    idxs_ap=idx_t[:, ci * CHUNK * 8:
                                                  ci * CHUNK * 8 + nblk * 8],
                                    num_idxs=nblk * P,
                                    num_idxs_reg=(nir_full if nblk == CHUNK
                                                  else nir_rem),
                                    elem_size=CH, single_packet=False)
                                msg_tiles[ci] = mt
                            mt = msg_tiles[ci]
                            nc.tensor.matmul(
                                h_ps[:],
                                sel_t[:, col * P:(col + 1) * P],
                                mt[:, wi * CH:(wi + 1) * CH],
                                start=(k == 0), stop=(k == kgrp[gi] - 1))
                            col += 1
                        pr_bf = prp.tile([P, CH], BF, tag="pr")
                        last_copy = nc.scalar.activation(
                            pr_bf[:], h_ps[:],
                            mybir.ActivationFunctionType.Copy)
                        last_store = nc.sync.dma_start(
                            rs_in[half][gx * P:(gx + 1) * P, :], pr_bf[:])
                        if gi == NGA - 1:
                            nc.gpsimd.collective_compute(
                                "ReduceScatter", mybir.AluOpType.add,
                                replica_groups=[list(range(NCORES))],
                                ins=[rs_in[0][:].opt()],
                                outs=[rs_out[0][:].opt()])
                    nc.gpsimd.collective_compute(
                        "ReduceScatter", mybir.AluOpType.add,
                        replica_groups=[list(range(NCORES))],
                        ins=[rs_in[1][:].opt()], outs=[rs_out[1][:].opt()])

                    # --- hV + bias for own nodes (overlaps RS_b)
                    hv_sb = hvp.tile([P, NBL * CH], BF, tag="hv")
                    for b in range(NBL):
                        v_ps = pshv.tile([P, CH], F32, tag="hv")
                        for g in range(kg):
                            nc.tensor.matmul(
                                v_ps[:],
                                lhsT_t[:, g * NPAD + b * P:g * NPAD + (b + 1) * P],
                                vt[:, g * CH:(g + 1) * CH],
                                start=(g == 0), stop=(g == kg - 1))
                        nc.vector.tensor_add(
                            v_ps[:], v_ps[:], ball_t[:, l * CH:(l + 1) * CH])
                        nc.vector.tensor_copy(hv_sb[:, b * CH:(b + 1) * CH],
                                              v_ps[:])

                    # --- epilogue: h' = relu(agg + hV + b); transpose to hT;
                    #     then immediately m(l+1)[b] = h'[b] @ W(l+1) so the
                    #     next layer's m fills the RS_b window
                    hT_next = htp.tile([P, KGC * NPAD], BF, tag="hT")
                    if l < NLAYERS - 1:
                        m_next = dram.tile([NPAD, CH], F8, tag=f"m{l + 1}")
                    for b in range(NBL):
                        half = int(b >= SPLIT_B)
                        bx = b - SPLIT_B if half else b
                        agg_t = aggp.tile([P, CH], BF, tag="agg_in")
                        i_ld = nc.sync.dma_start(
                            agg_t[:], rs_out[half][bx * P:(bx + 1) * P, :])
                        h_bf = hp.tile([P, CH], BF, tag="h")
                        i_ad = nc.vector.tensor_add(
                            h_bf[:], agg_t[:], hv_sb[:, b * CH:(b + 1) * CH])
                        i_rl = nc.scalar.activation(
                            h_bf[:], h_bf[:], mybir.ActivationFunctionType.Relu)
                        if b == 0:
                            # scheduler ordering hints: keep the epilogue
                            # behind the second half's copies/stores so it
                            # can't head-of-line-block engines on RS_a
                            add_dep_helper(i_ld.ins, last_store.ins,
                                           reason="epilogue after h2 stores")
                            add_dep_helper(i_ad.ins, last_copy.ins,
                                           reason="epilogue after h2 copies")
                            add_dep_helper(i_rl.ins, last_copy.ins,
                                           reason="epilogue after h2 copies")
                        for cg in range(KGC):
                            tr_ps = pstr.tile([P, P], BF, tag="tr")
                            i_tr = nc.tensor.transpose(
                                tr_ps[:], h_bf[:, cg * P:(cg + 1) * P], ident[:])
                            if b == 0 and cg == 0:
                                add_dep_helper(i_tr.ins, last_copy.ins,
                                               reason="epilogue after h2")
                            nc.vector.tensor_copy(
                                hT_next[:, cg * NPAD + b * P:cg * NPAD + (b + 1) * P],
                                tr_ps[:])
                        if l < NLAYERS - 1:
                            m_ps = psm.tile([P, CH], F32, tag="m")
                            for g in range(KGC):
                                nc.tensor.matmul(
                                    m_ps[:],
                                    hT_next[:, g * NPAD + b * P:
                                            g * NPAD + (b + 1) * P],
                                    wtn[:, g * CH:(g + 1) * CH],
                                    start=(g == 0), stop=(g == KGC - 1))
                            m_bf = mp.tile([P, CH], F8, tag="mbf")
                            nc.vector.tensor_copy(m_bf[:], m_ps[:])
                            nc.sync.dma_start(m_next[b * P:(b + 1) * P, :],
                                              m_bf[:])
                        else:
                            # final dense per block: logits = h7 @ Wd + bd,
                            # so blocks 0..SPLIT_B-1 overlap the last RS_b
                            o_sb = op.tile([P, N_LABELS], F32, tag="o")
                            fps = []
                            for c in range(3):
                                fin_ps = psagg.tile([P, FIN_CHUNK], F32,
                                                    tag="agg")
                                fps.append(fin_ps)
                            for g in range(KGC):
                                for c in range(3):
                                    nc.tensor.matmul(
                                        fps[c][:],
                                        hT_next[:, g * NPAD + b * P:
                                                g * NPAD + (b + 1) * P],
                                        wd_t[:, g * N_LABELS + c * FIN_CHUNK:
                                             g * N_LABELS + (c + 1) * FIN_CHUNK],
                                        start=(g == 0), stop=(g == KGC - 1))
                            for c in range(3):
                                sl = slice(c * FIN_CHUNK, (c + 1) * FIN_CHUNK)
                                nc.vector.tensor_add(fps[c][:], fps[c][:],
                                                     bdr_t[:, sl])
                                nc.scalar.activation(
                                    o_sb[:, sl], fps[c][:],
                                    mybir.ActivationFunctionType.Copy)
                            if rep == repeat - 1:
                                nc.sync.dma_start(out[b * P:(b + 1) * P, :],
                                                  o_sb[:])
                    hT_cur = hT_next
                    if l < NLAYERS - 1:
                        m_dram = m_next
                        vt = vt_t[:, :]

    _split_excess_waits(nc)
    lower_extended_insts(nc)
    return nc


# ------------------------------------------------------------- entry point
def kernel(x, src, dst, W1, V1, b1, Wk, Vk, bk, Wd, bd, _repeat=1, _nc_cache={}):
    x = np.asarray(x, np.float32)
    kgrp, idx_tabs, sel_tabs, newrow = _prep_edges(src, dst)

    key = (tuple(kgrp), _repeat)
    if key not in _nc_cache:
        _nc_cache[key] = _build(kgrp, repeat=_repeat)
    nc = _nc_cache[key]

    # weights (replicated, host-packed)
    w1p = _pack_rhs(np.asarray(W1, np.float32), KG1, CH).astype(BFNP)
    v1p = _pack_rhs(np.asarray(V1, np.float32), KG1, CH).astype(BFNP)
    wkp = np.concatenate(
        [_pack_rhs(np.asarray(Wk[i], np.float32), KGC, CH) for i in range(6)],
        axis=1).astype(BFNP)
    vkp = np.concatenate(
        [_pack_rhs(np.asarray(Vk[i], np.float32), KGC, CH) for i in range(6)],
        axis=1).astype(BFNP)
    wdp = _pack_rhs(np.asarray(Wd, np.float32), KGC, N_LABELS).astype(BFNP)
    ballv = np.concatenate(
        [np.asarray(b1, np.float32)] + [np.asarray(bk[i], np.float32)
                                        for i in range(6)])
    ballp = np.broadcast_to(ballv, (P, NLAYERS * CH)).copy()
    bdp = np.broadcast_to(np.asarray(bd, np.float32), (P, N_LABELS)).copy()

    in_maps = []
    for p in range(NCORES):
        xp = np.zeros((NPAD, IN_F), np.float32)
        nodes = np.arange(p * NPC, (p + 1) * NPC)
        xp[newrow[nodes]] = x[nodes]
        xTp = _pack_lhsT(np.ascontiguousarray(xp.T), KG1).astype(BFNP)
        in_maps.append({
            "xT": xTp, "idx": idx_tabs[p], "sel": sel_tabs[p],
            "w1": w1p, "v1": v1p, "wk": wkp, "vk": vkp, "wd": wdp,
            "ball": ballp, "bdr": bdp,
        })

    res = run_bass_kernel_spmd(nc, in_maps, core_ids=list(range(NCORES)))
    outp = np.empty((N_NODES, N_LABELS), np.float32)
    for p in range(NCORES):
        nodes = np.arange(p * NPC, (p + 1) * NPC)
        outp[nodes] = res.results[p]["out"][newrow[nodes]]
    return outp


# revision 39
# speedup vs baseline: 3.1208x; 1.0053x over previous
"""ARMA-style GNN message passing on 8 TRN2 NeuronCores.

Reference computation (per layer, 7 layers):
    m   = h @ W                                  [N, CH]
    agg = segment_sum(w[:,None] * m[dst], src)   [N, CH]
    h'  = relu(agg + h @ V + b)
then logits = h @ Wd + bd.

Strategy (dst-partitioned edges + ReduceScatter):
  - 8 cores own 1250 nodes each (padded to 1280 = 10 blocks of 128).
  - Edge (s, d) is processed by the core owning d: the message row m[d]
    lives in that core's local m table, so gathers are local (no
    AllGather of m). Messages are scatter-added into per-src-block
    partial aggregates via PE matmuls with host-built bf16 "selection"
    matrices (segment-sum == sel.T @ msg_rows).
  - Host sorts each core's edges by global src block (80 blocks of 128
    padded-global rows); per (core, src block) the edge count is padded
    to a multiple of 128; the per-block count is the max over cores so
    the SPMD program is identical.
  - Per layer: m = h @ W for own nodes (PE) -> staged to local DRAM;
    batched dma_gather calls (CHUNK edge blocks each, ~1KB rows) pull
    per-edge message rows; per src block the sel matmuls accumulate
    into PSUM; partials staged to DRAM; one ReduceScatter sums the
    [80*128, CH] partials and hands each core its own 10 blocks.
    h@V + bias runs on PE during the ReduceScatter; epilogue adds
    agg + hV, applies relu, and PE-transposes into the next layer's
    stationary operand hT.
  - Final dense layer per core; host concatenates.

All matmuls run in bf16 with fp32 PSUM accumulation.
"""
import numpy as np
import ml_dtypes

import concourse.bass as bass
import concourse.tile as tile
from concourse.tile import add_dep_helper
import concourse.mybir as mybir
from concourse.vector_clock import ScopedClock
from concourse.bass_utils import run_bass_kernel_spmd
from concourse.masks import make_identity
from concourse import library_config
from concourse.library_overlay import lower_extended_insts

# ---------------------------------------------------------------- constants
N_NODES = 10000
N_EDGES = 160000
IN_F = 256
CH = 512
N_LABELS = 1440
NCORES = 8
NPC = N_NODES // NCORES      # 1250 nodes per core
P = 128
NBL = 10                     # node blocks per core (10*128 = 1280)
NPAD = NBL * P               # padded nodes per core
NSB = NCORES * NBL           # 80 global src blocks
NLAYERS = 7
KG1 = IN_F // P              # 2 contraction blocks in layer 1
KGC = CH // P                # 4 contraction blocks in layers 2..7
FIN_CHUNK = 480              # 1440 = 3 * 480, fits one PSUM bank in f32
SPLIT_B = 6                  # node blocks per core in the first RS half
CHUNK = 8                    # edge blocks per dma_gather call

BF = mybir.dt.bfloat16
F8 = mybir.dt.float8e4          # e4m3: message table / gather payload
F32 = mybir.dt.float32
BFNP = ml_dtypes.bfloat16


# ------------------------------------------------------- walrus workarounds
def _patched_drain_and_barrier(self, tick_clock, wait_clock):
    # This walrus build rejects >1-2 sync waits on one TPB_CTRL; put the
    # kernel-tail drain's waits on separate preceding SP nops instead.
    nc = self.nc
    probe = nc.sync.nop(nofuse=True, hint="drain_waits")
    wait_clock.add_sem_waits(probe.ins, ScopedClock({None: tick_clock.global_clock}))
    si = probe.ins.sync_info
    waits = list(si.on_wait) if si is not None else []
    if len(waits) > 1:
        si.on_wait = waits[:1]
        for i in range(1, len(waits)):
            n2 = nc.sync.nop(nofuse=True, hint=f"drain_waits_{i}")
            n2.ins.sync_info = mybir.SyncInfo(on_wait=[waits[i]], on_update=[])
    nc.sync.drain()
    nc.all_engine_barrier()
    assert self.sems is not None
    popped = nc._tile_sem_poison_stack.pop()
    assert popped is self._sem_poison
    nc.clear_and_free_semaphores(list(self.sems.allocated().values()))
    nc.all_engine_barrier()


tile.TileContext._drain_and_barrier = _patched_drain_and_barrier


def _split_excess_waits(nc, limit=1):
    # Same ISA restriction for ordinary instructions: hoist excess sync
    # waits onto injected same-engine nops placed just before.
    for func in nc.m.functions:
        for bb in func.blocks:
            out = []
            for ins in bb.instructions:
                si = ins.sync_info
                if si is not None and si.on_wait and len(si.on_wait) > limit:
                    waits = list(si.on_wait)
                    excess, keep = waits[:-limit], waits[-limit:]
                    for i in range(0, len(excess), limit):
                        out.append(mybir.InstNoOp(
                            name=f"{ins.name}_xw{i}",
                            engine=ins.engine,
                            ins=[], outs=[],
                            sync_info=mybir.SyncInfo(
                                on_wait=excess[i:i + limit], on_update=[]),
                        ))
                    si.on_wait = keep
                out.append(ins)
            bb.instructions[:] = out


# ------------------------------------------------------------- host prep
def _balance_rows(src, dst):
    """Permute rows within each core so every (src block, dst core) edge
    count stays near/below 256 -> fewer padded edge blocks. Returns
    newrow[node] in [0, NPAD) (block-aligned; rows >= count are padding)."""
    core_of = np.arange(N_NODES) // NPC
    dstcore = core_of[dst]
    e = np.zeros((N_NODES, NCORES), np.int32)
    np.add.at(e, (src, dstcore), 1)
    newrow = np.full(N_NODES, -1, np.int64)
    for a in range(NCORES):
        nodes = np.arange(a * NPC, (a + 1) * NPC)
        ev = e[nodes].astype(np.float64)
        order = np.argsort(-ev.sum(1), kind='stable')
        L = np.zeros((NBL, NCORES))
        cntg = np.zeros(NBL, np.int64)
        asg = np.zeros(NPC, np.int64)
        CAP = 253.0
        for idx in order:
            cand = ev[idx]
            best, bestpen = -1, None
            for g in range(NBL):
                if cntg[g] >= P:
                    continue
                newL = L[g] + cand
                pen = (np.maximum(0, newL - CAP).sum(), newL.max(), cntg[g])
                if bestpen is None or pen < bestpen:
                    bestpen, best = pen, g
            L[best] += cand
            asg[idx] = best
            cntg[best] += 1
        rng = np.random.default_rng(a)
        for it in range(4000):
            g1 = int(np.argmax(L.max(1))) if rng.random() < 0.7 \
                else int(rng.integers(NBL))
            q = int(np.argmax(L[g1]))
            if L[g1][q] <= 256:
                break
            members = np.where(asg == g1)[0]
            i1 = members[np.argmax(ev[members, q])]
            room = np.where(cntg < P)[0]
            moved = False
            if len(room):
                g2 = room[np.argmin(L[room, q])]
                if g2 != g1 and (L[g2] + ev[i1]).max() <= 256:
                    L[g1] -= ev[i1]; L[g2] += ev[i1]
                    cntg[g1] -= 1; cntg[g2] += 1; asg[i1] = g2
                    moved = True
            if not moved:
                g2 = int(np.argmin(L[:, q]))
                mem2 = np.where(asg == g2)[0]
                i2 = mem2[np.argmin(ev[mem2, q])]
                d1, d2 = ev[i1], ev[i2]
                nl1, nl2 = L[g1] - d1 + d2, L[g2] - d2 + d1
                if max(nl1.max(), nl2.max()) < max(L[g1].max(), L[g2].max()):
                    L[g1], L[g2] = nl1, nl2
                    asg[i1], asg[i2] = g2, g1
        fill = np.zeros(NBL, np.int64)
        for j in range(NPC):
            g = asg[j]
            newrow[nodes[j]] = g * P + fill[g]
            fill[g] += 1
    return newrow


def _prep_edges(src, dst):
    """Partition edges by owning core of dst; per core group edges by
    global src block, reordered so each core's first 5 node blocks come
    first (split ReduceScatter halves). Returns (kgrp, idx_tabs, sel_tabs):
      kgrp[g]    edge-block count of src group g in split order
      idx_tabs[p] int16 [128, NEB*8] 16-wrapped local m-row gather indices
      sel_tabs[p] bf16 [128, NEB*128] selection/weight matrices
    """
    src = np.asarray(src).astype(np.int64)
    dst = np.asarray(dst).astype(np.int64)
    deg_out = np.maximum(np.bincount(src, minlength=N_NODES), 1.0).astype(np.float32)
    deg_in = np.maximum(np.bincount(dst, minlength=N_NODES), 1.0).astype(np.float32)
    w = (1.0 / np.sqrt(deg_out[src] * deg_in[dst])).astype(np.float32)

    newrow = _balance_rows(src, dst)        # in-core row permutation
    core = dst // NPC                       # owning core (dst side)
    lrow = newrow[dst]                      # gather row in local m table
    g_src = (src // NPC) * NPAD + newrow[src]
    sb = g_src // P                         # global src block 0..79
    scol = g_src % P                        # column within sel matrix

    order = np.lexsort((sb, core))
    core_s, sb_s = core[order], sb[order]
    lrow_s, scol_s, w_s = lrow[order], scol[order], w[order]
    counts = np.zeros((NCORES, NSB), np.int64)
    np.add.at(counts, (core_s, sb_s), 1)
    # split order: group g -> src block: first all (q, b<SPLIT_B), rest after
    GORDER = ([q * NBL + b for q in range(NCORES) for b in range(SPLIT_B)]
              + [q * NBL + b for q in range(NCORES)
                 for b in range(SPLIT_B, NBL)])
    kgrp = [max(1, int(-(-counts[:, g].max() // P))) for g in GORDER]
    neb = sum(kgrp)

    idx_tabs, sel_tabs = [], []
    starts = np.zeros((NCORES, NSB), np.int64)
    flat = counts.ravel().cumsum()
    starts.ravel()[1:] = flat[:-1]
    for p in range(NCORES):
        flat_idx = np.zeros(neb * P, np.int64)      # slot -> local m row
        sel_t = np.zeros((P, neb * P), np.float32)
        col = 0
        for gi, b in enumerate(GORDER):
            s0, cnt = starts[p, b], counts[p, b]
            g = lrow_s[s0:s0 + cnt]
            c = scol_s[s0:s0 + cnt]
            ww = w_s[s0:s0 + cnt]
            for k in range(kgrp[gi]):
                lo, hi = k * P, min((k + 1) * P, cnt)
                if hi > lo:
                    lanes = np.arange(hi - lo)
                    flat_idx[col * P + lanes] = g[lo:hi]
                    sel_t[lanes, col * P + c[lo:hi]] = ww[lo:hi]
                col += 1
        # 16-wrap per gather call of CHUNK blocks: slot i of call c lands
        # at [i%16, c*CHUNK*8 + i//16], replicated across the 8 Q7 cores'
        # partition groups (each Q7 core reads its own 16 partitions)
        idx_t = np.zeros((P, neb * 8), np.int16)
        gidx = np.arange(neb * P)
        call = gidx // (CHUNK * P)
        i_in = gidx % (CHUNK * P)
        for q7 in range(8):
            idx_t[16 * q7 + i_in % 16, call * CHUNK * 8 + i_in // 16] = flat_idx
        idx_tabs.append(idx_t)
        sel_tabs.append(sel_t.astype(BFNP))
    return kgrp, idx_tabs, sel_tabs, newrow


def _pack_lhsT(xT, kg):
    """[kg*128, NPAD] -> [128, kg*NPAD] (partition-major kg blocks)."""
    return np.ascontiguousarray(
        xT.reshape(kg, P, NPAD).transpose(1, 0, 2).reshape(P, kg * NPAD))


def _pack_rhs(Wm, kg, n):
    """[kg*128, n] -> [128, kg*n]."""
    return np.ascontiguousarray(
        Wm.reshape(kg, P, n).transpose(1, 0, 2).reshape(P, kg * n))


# ------------------------------------------------------------- device build
def _build(kgrp, repeat=1):
    neb = sum(kgrp)
    ncalls = -(-neb // CHUNK)
    nc = bass.Bass("TRN2", target_bir_lowering=False, debug=False,
                   num_devices=NCORES)

    def din(name, shape, dt):
        return nc.dram_tensor(name, shape, dt, kind="ExternalInput").ap()

    xT = din("xT", [P, KG1 * NPAD], BF)
    idx = din("idx", [P, neb * 8], mybir.dt.int16)
    sel = din("sel", [P, neb * P], BF)
    w1 = din("w1", [P, KG1 * CH], BF)
    v1 = din("v1", [P, KG1 * CH], BF)
    wk = din("wk", [P, 6 * KGC * CH], BF)
    vk = din("vk", [P, 6 * KGC * CH], BF)
    wd = din("wd", [P, KGC * N_LABELS], BF)
    ball = din("ball", [P, NLAYERS * CH], F32)
    bdr = din("bdr", [P, N_LABELS], F32)
    out = nc.dram_tensor("out", [NPAD, N_LABELS], F32, kind="ExternalOutput").ap()

    with tile.TileContext(nc) as tc:
        with (
            tc.tile_pool(name="const", bufs=1) as cp,
            tc.tile_pool(name="wv", bufs=2) as wvp,
            tc.tile_pool(name="ht", bufs=2) as htp,
            tc.tile_pool(name="mout", bufs=4) as mp,
            tc.tile_pool(name="msg", bufs=4) as msgp,
            tc.tile_pool(name="part", bufs=10) as prp,
            tc.tile_pool(name="hvs", bufs=2) as hvp,
            tc.tile_pool(name="aggin", bufs=3) as aggp,
            tc.tile_pool(name="hact", bufs=3) as hp,
            tc.tile_pool(name="outs", bufs=2) as op,
            tc.tile_pool(name="psm", bufs=2, space="PSUM") as psm,
            tc.tile_pool(name="psagg", bufs=3, space="PSUM") as psagg,
            tc.tile_pool(name="pshv", bufs=1, space="PSUM") as pshv,
            tc.tile_pool(name="pstr", bufs=2, space="PSUM") as pstr,
            tc.tile_pool(name="dram", bufs=1, space="DRAM") as dram,
        ):
            nc.gpsimd.load_library(library_config.mlp)
            # ---- constants to SBUF
            xT_t = cp.tile([P, KG1 * NPAD], BF)
            nc.sync.dma_start(xT_t[:], xT[:])
            neb_h1 = sum(kgrp[:NCORES * SPLIT_B])
            c1 = -(-neb_h1 // CHUNK) * CHUNK * 8     # idx cols, call-aligned
            idx_t = cp.tile([P, neb * 8], mybir.dt.int16)
            nc.sync.dma_start(idx_t[:, :c1], idx[:, :c1])
            nc.sync.dma_start(idx_t[:, c1:], idx[:, c1:])
            sel_t = cp.tile([P, neb * P], BF)
            nc.sync.dma_start(sel_t[:, :neb_h1 * P], sel[:, :neb_h1 * P])
            nc.sync.dma_start(sel_t[:, neb_h1 * P:], sel[:, neb_h1 * P:])
            w1_t = cp.tile([P, KG1 * CH], BF)
            nc.sync.dma_start(w1_t[:], w1[:])
            v1_t = cp.tile([P, KG1 * CH], BF)
            nc.sync.dma_start(v1_t[:], v1[:])
            wd_t = cp.tile([P, KGC * N_LABELS], BF)
            nc.sync.dma_start(wd_t[:], wd[:])
            ball_t = cp.tile([P, NLAYERS * CH], F32)
            nc.sync.dma_start(ball_t[:], ball[:])
            bdr_t = cp.tile([P, N_LABELS], F32)
            nc.sync.dma_start(bdr_t[:], bdr[:])
            ident = cp.tile([P, P], BF)
            make_identity(nc, ident[:])

            # num_idxs registers for the batched gathers (to_reg on an int
            # immediate has no free-register pool under TileContext)
            rem = neb % CHUNK
            nir_full = nc.gpsimd.alloc_register("nir_full")
            nc.gpsimd.reg_mov(nir_full, CHUNK * P)
            nir_rem = None
            if rem:
                nir_rem = nc.gpsimd.alloc_register("nir_rem")
                nc.gpsimd.reg_mov(nir_rem, rem * P)

            for rep in range(repeat):
                # --- prologue: m(0) = x @ W1, staged to local DRAM (fp8)
                m_dram = dram.tile([NPAD, CH], F8, tag="m0")
                for b in range(NBL):
                    m_ps = psm.tile([P, CH], F32, tag="m")
                    for g in range(KG1):
                        nc.tensor.matmul(
                            m_ps[:],
                            xT_t[:, g * NPAD + b * P:g * NPAD + (b + 1) * P],
                            w1_t[:, g * CH:(g + 1) * CH],
                            start=(g == 0), stop=(g == KG1 - 1))
                    m_bf = mp.tile([P, CH], F8, tag="mbf")
                    nc.vector.tensor_copy(m_bf[:], m_ps[:])
                    nc.sync.dma_start(m_dram[b * P:(b + 1) * P, :], m_bf[:])

                hT_cur = None
                vt = v1_t[:, :]
                for l in range(NLAYERS):
                    kg = KG1 if l == 0 else KGC
                    lhsT_t = xT_t if l == 0 else hT_cur
                    if l < NLAYERS - 1:
                        # weights for the NEXT layer: wtn feeds m(l+1) at the
                        # end of this layer; vtn feeds hv(l+1) next layer
                        wt_t = wvp.tile([P, KGC * CH], BF, tag="wk")
                        nc.sync.dma_start(
                            wt_t[:], wk[:, l * KGC * CH:(l + 1) * KGC * CH])
                        vt_t = wvp.tile([P, KGC * CH], BF, tag="vk")
                        nc.sync.dma_start(
                            vt_t[:], vk[:, l * KGC * CH:(l + 1) * KGC * CH])
                        wtn = wt_t[:, :]

                    # --- gathers (batched) + scatter matmuls, interleaved,
                    #     in two halves with a ReduceScatter per half so the
                    #     first collective overlaps the second half's work
                    NGA = NCORES * SPLIT_B
                    NGB = NSB - NGA
                    rs_in_a = dram.tile([NGA * P, CH], BF, tag=f"ri{l}h0")
                    rs_in_b = dram.tile([NGB * P, CH], BF, tag=f"ri{l}h1")
                    rs_out_a = dram.tile([SPLIT_B * P, CH], BF, tag=f"ro{l}h0")
                    rs_out_b = dram.tile([(NBL - SPLIT_B) * P, CH], BF,
                                         tag=f"ro{l}h1")
                    rs_in = [rs_in_a, rs_in_b]
                    rs_out = [rs_out_a, rs_out_b]
                    msg_tiles = [None] * ncalls
                    col = 0
                    for gi in range(NSB):
                        half = int(gi >= NGA)
                        gx = gi - NGA if half else gi
                        h_ps = psagg.tile([P, CH], F32, tag="agg")
                        for k in range(kgrp[gi]):
                            ci, wi = col // CHUNK, col % CHUNK
                            if wi == 0:
                                nblk = min(CHUNK, neb - ci * CHUNK)
                                mt = msgp.tile([P, CHUNK * CH], F8, tag="msg")
                                out3 = mt[:].rearrange(
                                    "p (k c) -> p k c", c=CH)[:, :nblk, :]
                                nc.gpsimd.dma_gather(
                                    out_ap=out3,
                                    in_ap=m_dram[:, :],
                                    idxs_ap=idx_t[:, ci * CHUNK * 8:
                                                  ci * CHUNK * 8 + nblk * 8],
                                    num_idxs=nblk * P,
                                    num_idxs_reg=(nir_full if nblk == CHUNK
                                                  else nir_rem),
                                    elem_size=CH, single_packet=False)
                                msg_tiles[ci] = mt
                            mt = msg_tiles[ci]
                            nc.tensor.matmul(
                                h_ps[:],
                                sel_t[:, col * P:(col + 1) * P],
                                mt[:, wi * CH:(wi + 1) * CH],
                                start=(k == 0), stop=(k == kgrp[gi] - 1))
                            col += 1
                        pr_bf = prp.tile([P, CH], BF, tag="pr")
                        last_copy = nc.scalar.activation(
                            pr_bf[:], h_ps[:],
                            mybir.ActivationFunctionType.Copy)
                        last_store = nc.sync.dma_start(
                            rs_in[half][gx * P:(gx + 1) * P, :], pr_bf[:])
                        if gi == NGA - 1:
                            nc.gpsimd.collective_compute(
                                "ReduceScatter", mybir.AluOpType.add,
                                replica_groups=[list(range(NCORES))],
                                ins=[rs_in[0][:].opt()],
                                outs=[rs_out[0][:].opt()])
                    nc.gpsimd.collective_compute(
                        "ReduceScatter", mybir.AluOpType.add,
                        replica_groups=[list(range(NCORES))],
                        ins=[rs_in[1][:].opt()], outs=[rs_out[1][:].opt()])

                    # --- hV + bias for own nodes (overlaps RS_b)
                    hv_sb = hvp.tile([P, NBL * CH], BF, tag="hv")
                    for b in range(NBL):
                        v_ps = pshv.tile([P, CH], F32, tag="hv")
                        for g in range(kg):
                            nc.tensor.matmul(
                                v_ps[:],
                                lhsT_t[:, g * NPAD + b * P:g * NPAD + (b + 1) * P],
                                vt[:, g * CH:(g + 1) * CH],
                                start=(g == 0), stop=(g == kg - 1))
                        nc.vector.tensor_add(
                            v_ps[:], v_ps[:], ball_t[:, l * CH:(l + 1) * CH])
                        nc.vector.tensor_copy(hv_sb[:, b * CH:(b + 1) * CH],
                                              v_ps[:])

                    # --- epilogue: h' = relu(agg + hV + b); transpose to hT;
                    #     then immediately m(l+1)[b] = h'[b] @ W(l+1) so the
                    #     next layer's m fills the RS_b window
                    hT_next = htp.tile([P, KGC * NPAD], BF, tag="hT")
                    if l < NLAYERS - 1:
                        m_next = dram.tile([NPAD, CH], F8, tag=f"m{l + 1}")
                    for b in range(NBL):
                        half = int(b >= SPLIT_B)
                        bx = b - SPLIT_B if half else b
                        agg_t = aggp.tile([P, CH], BF, tag="agg_in")
                        i_ld = nc.sync.dma_start(
                            agg_t[:], rs_out[half][bx * P:(bx + 1) * P, :])
                        h_bf = hp.tile([P, CH], BF, tag="h")
                        i_ad = nc.vector.tensor_add(
                            h_bf[:], agg_t[:], hv_sb[:, b * CH:(b + 1) * CH])
                        i_rl = nc.scalar.activation(
                            h_bf[:], h_bf[:], mybir.ActivationFunctionType.Relu)
                        if b == 0:
                            # scheduler ordering hints: keep the epilogue
                            # behind the second half's copies/stores so it
                            # can't head-of-line-block engines on RS_a
                            add_dep_helper(i_ld.ins, last_store.ins,
                                           reason="epilogue after h2 stores")
                            add_dep_helper(i_ad.ins, last_copy.ins,
                                           reason="epilogue after h2 copies")
                            add_dep_helper(i_rl.ins, last_copy.ins,
                                           reason="epilogue after h2 copies")
                        for cg in range(KGC):
                            tr_ps = pstr.tile([P, P], BF, tag="tr")
                            i_tr = nc.tensor.transpose(
                                tr_ps[:], h_bf[:, cg * P:(cg + 1) * P], ident[:])
                            if b == 0 and cg == 0:
                                add_dep_helper(i_tr.ins, last_copy.ins,
                                               reason="epilogue after h2")
                            nc.vector.tensor_copy(
                                hT_next[:, cg * NPAD + b * P:cg * NPAD + (b + 1) * P],
                                tr_ps[:])
                        if l < NLAYERS - 1:
                            m_ps = psm.tile([P, CH], F32, tag="m")
                            for g in range(KGC):
                                nc.tensor.matmul(
                                    m_ps[:],
                                    hT_next[:, g * NPAD + b * P:
                                            g * NPAD + (b + 1) * P],
                                    wtn[:, g * CH:(g + 1) * CH],
                                    start=(g == 0), stop=(g == KGC - 1))
                            m_bf = mp.tile([P, CH], F8, tag="mbf")
                            nc.vector.tensor_copy(m_bf[:], m_ps[:])
                            nc.sync.dma_start(m_next[b * P:(b + 1) * P, :],
                                              m_bf[:])
                        else:
                            # final dense per block: logits = h7 @ Wd + bd,
                            # so blocks 0..SPLIT_B-1 overlap the last RS_b
                            o_sb = op.tile([P, N_LABELS], F32, tag="o")
                            fps = []
                            for c in range(3):
                                fin_ps = psagg.tile([P, FIN_CHUNK], F32,
                                                    tag="agg")
                                fps.append(fin_ps)
                            for g in range(KGC):
                                for c in range(3):
                                    nc.tensor.matmul(
                                        fps[c][:],
                                        hT_next[:, g * NPAD + b * P:
                                                g * NPAD + (b + 1) * P],
                                        wd_t[:, g * N_LABELS + c * FIN_CHUNK:
                                             g * N_LABELS + (c + 1) * FIN_CHUNK],
                                        start=(g == 0), stop=(g == KGC - 1))
                            for c in range(3):
                                sl = slice(c * FIN_CHUNK, (c + 1) * FIN_CHUNK)
                                nc.vector.tensor_add(fps[c][:], fps[c][:],
                                                     bdr_t[:, sl])
                                nc.scalar.activation(
                                    o_sb[:, sl], fps[c][:],
                                    mybir.ActivationFunctionType.Copy)
                            if rep == repeat - 1:
                                nc.sync.dma_start(out[b * P:(b + 1) * P, :],
                                                  o_sb[:])
                    hT_cur = hT_next
                    if l < NLAYERS - 1:
                        m_dram = m_next
                        vt = vt_t[:, :]

    _split_excess_waits(nc)
    lower_extended_insts(nc)
    return nc


# ------------------------------------------------------------- entry point
def kernel(x, src, dst, W1, V1, b1, Wk, Vk, bk, Wd, bd, _repeat=1, _nc_cache={}):
    x = np.asarray(x, np.float32)
    kgrp, idx_tabs, sel_tabs, newrow = _prep_edges(src, dst)

    key = (tuple(kgrp), _repeat)
    if key not in _nc_cache:
        _nc_cache[key] = _build(kgrp, repeat=_repeat)
    nc = _nc_cache[key]

    # weights (replicated, host-packed)
    w1p = _pack_rhs(np.asarray(W1, np.float32), KG1, CH).astype(BFNP)
    v1p = _pack_rhs(np.asarray(V1, np.float32), KG1, CH).astype(BFNP)
    wkp = np.concatenate(
        [_pack_rhs(np.asarray(Wk[i], np.float32), KGC, CH) for i in range(6)],
        axis=1).astype(BFNP)
    vkp = np.concatenate(
        [_pack_rhs(np.asarray(Vk[i], np.float32), KGC, CH) for i in range(6)],
        axis=1).astype(BFNP)
    wdp = _pack_rhs(np.asarray(Wd, np.float32), KGC, N_LABELS).astype(BFNP)
    ballv = np.concatenate(
        [np.asarray(b1, np.float32)] + [np.asarray(bk[i], np.float32)
                                        for i in range(6)])
    ballp = np.broadcast_to(ballv, (P, NLAYERS * CH)).copy()
    bdp = np.broadcast_to(np.asarray(bd, np.float32), (P, N_LABELS)).copy()

    in_maps = []
    for p in range(NCORES):
        xp = np.zeros((NPAD, IN_F), np.float32)
        nodes = np.arange(p * NPC, (p + 1) * NPC)
        xp[newrow[nodes]] = x[nodes]
        xTp = _pack_lhsT(np.ascontiguousarray(xp.T), KG1).astype(BFNP)
        in_maps.append({
            "xT": xTp, "idx": idx_tabs[p], "sel": sel_tabs[p],
            "w1": w1p, "v1": v1p, "wk": wkp, "vk": vkp, "wd": wdp,
            "ball": ballp, "bdr": bdp,
        })

    res = run_bass_kernel_spmd(nc, in_maps, core_ids=list(range(NCORES)))
    outp = np.empty((N_NODES, N_LABELS), np.float32)
    for p in range(NCORES):
        nodes = np.arange(p * NPC, (p + 1) * NPC)
        outp[nodes] = res.results[p]["out"][newrow[nodes]]
    return outp
